# revision 19
# baseline (speedup 1.0000x reference)
"""Trainium2 Bass kernel for nn_BasicBlock (conv-SE-prune-BN residual block).

Data-parallel over batch across 8 NeuronCores; b_loc = 1024 per core.

v3 design (vs baseline): single x load, everything SBUF-resident.
 - Host pre-transposes x to [C, b_loc, 8, 8] and casts to bf16; output is
   returned bf16 [C, b_loc, 8, 8] and cast back on host.
 - Samples are split into two partition halves: batch 0:512 lives on
   partitions 0:64 ("L"), batch 512:1024 on 64:128 ("H"), giving 128-wide
   elementwise ops. Conv groups alternate L/H; the conv lhs has an L and
   an H variant (output accumulator halves swapped) so conv outputs land
   on their home partitions.
 - Conv: 3 matmuls per 6-sample tile, K=128 = channels + flat-shifted
   duplicate, M=128 = two accumulators (A, B). A evacuated by ACT/DVE
   copy psum->SBUF bf16; B evacuated to a bounce buffer and merged into
   R by a gpsimd DMA with accum_op=add (CCE inline add, crosses
   partitions for free).
 - Padded layout per sample is 10 rows x 10 cols, image at rows 1:9,
   cols 2:10 (so interior rows are 4-byte aligned for DVE 2x/4x modes).
   rhs views take cols 1:10; A-half out = view cols [0:8], B = [1:9].
 - Pooling from the resident bf16 copy (pair-packed, 128 partitions),
   fc gates computed pair-packed via block-packed fc weights, AllGather,
   then threshold bisection (14 iters) on a 1/8 subsample, all
   interleaved with conv1 groups. P3a (gate apply + BN1 stats) also
   interleaves with conv1's back half.
 - BN stats: per-group accum_out columns, reduced + partition-folded
   (gpsimd DMA add) + AllReduduced; coefs duplicated to both halves.
 - P5 (bn2 affine + residual + relu) runs pair-packed from SBUF.
"""
import numpy as np

import concourse.bacc as bacc
import concourse.bass as bass
import concourse.mybir as mybir
import concourse.tile as tile

F32 = mybir.dt.float32
BF16 = mybir.dt.bfloat16
I32 = mybir.dt.int32
AF = mybir.ActivationFunctionType
ALU = mybir.AluOpType
AX = mybir.AxisListType

C = 64
HW = 64
TILE_B = 6
GB = 18          # samples per conv group
PRUNE_RATE = 0.2
EPS = 1e-5
BISECT_ITERS = 12
SUB = 64         # bisect subsample columns per (core, partition)
PR, PW = 10, 10  # padded rows / cols per sample
PADSZ = PR * PW


def _pairs(half):
    """[(j, s0, ns)] covering one 512-sample half by 18-sample groups."""
    out = []
    s0 = 0
    j = 0
    while s0 < half:
        ns = min(GB, half - s0)
        out.append((j, s0, ns))
        s0 += ns
        j += 1
    return out


def _tiles(ns):
    t, b0 = [], 0
    while b0 < ns:
        nb = min(TILE_B, ns - b0)
        t.append((b0, nb))
        b0 += nb
    return t


def _transpose64(nc, dst_ap, src_ap):
    for i in (0, 32):
        for j in (0, 32):
            nc.vector.transpose(out=dst_ap[j:j + 32, i:i + 32],
                                in_=src_ap[i:i + 32, j:j + 32])


def build_nc(n_cores, b_loc):
    B_glob = n_cores * b_loc
    HALF = b_loc // 2
    N1 = float(B_glob * HW)
    n_sub = n_cores * 128 * SUB
    k_sub = PRUNE_RATE * n_sub
    D0s = float(2.0 * k_sub - n_sub)
    rg = [list(range(n_cores))]

    pairs = _pairs(HALF)          # 29 pairs
    NP = len(pairs)
    NG = 2 * NP                   # 58 groups, order L0 H0 L1 H1 ...

    nc = bacc.Bacc("TRN2", target_bir_lowering=False, debug=False,
                   enable_asserts=True, num_devices=n_cores)

    x_in = nc.dram_tensor("x", [C, b_loc, PR, PW], BF16,
                          kind="ExternalInput")
    xr_in = nc.dram_tensor("xr", [C, b_loc, 8, 8], BF16,
                           kind="ExternalInput")
    lhsw_in = nc.dram_tensor("lhsw", [128, 2, 2, 3, 128], BF16,
                             kind="ExternalInput")
    fc1t_in = nc.dram_tensor("fc1t", [128, 32], F32, kind="ExternalInput")
    fc2t_in = nc.dram_tensor("fc2t", [32, 128], F32, kind="ExternalInput")
    fc1bp_in = nc.dram_tensor("fc1bp", [32, 1], F32, kind="ExternalInput")
    fc2bp_in = nc.dram_tensor("fc2bp", [128, 1], F32, kind="ExternalInput")
    vecs_in = nc.dram_tensor("vecsp", [128, 4], F32, kind="ExternalInput")
    out_d = nc.dram_tensor("out", [C, b_loc, 8, 8], BF16, kind="ExternalOutput")

    with tile.TileContext(nc) as tc:
        with (
            tc.tile_pool(name="persist", bufs=1) as pp,
            tc.tile_pool(name="rings", bufs=2) as rp,
            tc.tile_pool(name="small", bufs=2) as smallp,
            tc.tile_pool(name="psc", bufs=2, space="PSUM") as psc,
            tc.tile_pool(name="psf", bufs=1, space="PSUM") as psf,
            tc.tile_pool(name="dram", bufs=1, space="DRAM") as dramp,
        ):
            # ---------------- persistent SBUF ----------------
            resid = pp.tile([128, HALF * HW], BF16, tag="resid")
            R = pp.tile([128, HALF * HW], BF16, tag="R")
            xpad = pp.tile([128, 3, GB, PR, PW], BF16, tag="xpad")
            ypad = pp.tile([128, 3, GB, PR, PW], BF16, tag="ypad")
            pooled = pp.tile([128, HALF], F32, tag="pooled")
            gates = pp.tile([128, HALF], F32, tag="gates")
            sep = pp.tile([128, HALF], BF16, tag="sep")
            gata_s = pp.tile([128, n_cores, SUB], BF16, tag="gata_s")
            onesKM = pp.tile([128, 128], BF16, tag="onesKM")
            stats = pp.tile([128, 192], F32, tag="stats")
            q2s = pp.tile([128, 32], F32, tag="q2s")
            vecs = pp.tile([128, 8], F32, tag="vecs")
            # vecs cols: 0=bn1_g 1=bn1_b 2=bn2_g 3=bn2_b
            fc1b = pp.tile([32, 1], F32, tag="fc1b")
            fc1T = pp.tile([128, 32], F32, tag="fc1T")
            fc2T = pp.tile([32, 128], F32, tag="fc2T")
            cf1 = pp.tile([128, 2], F32, tag="cf1")
            cf2 = pp.tile([128, 2], F32, tag="cf2")
            eps_t = pp.tile([C, 1], F32, tag="eps")
            lh = pp.tile([128, 2], F32, tag="lh")
            Tt = pp.tile([128, 1], F32, tag="Tt")
            negT = pp.tile([128, 1], F32, tag="negT")
            cjunk = pp.tile([128, n_cores * SUB], BF16, tag="cjunk")
            onesv = pp.tile([128, n_cores * SUB], BF16, tag="onesv")
            yst = pp.tile([128, 2, GB, PR, PW], BF16, tag="yst")
            scratch = pp.tile([C, 8], F32, tag="scratch")
            sqt = pp.tile([128, 4], F32, tag="sqt")

            xpad_f = xpad[:].rearrange("p s b r w -> p s (b r w)")
            ypad_f = ypad[:].rearrange("p s b r w -> p s (b r w)")

            # dram bounce buffers
            bar_sb = pp.tile([1, 1], F32, tag="bar_sb")
            bar_in = dramp.tile([1, 1], F32, tag="bar_in")
            bar_out = dramp.tile([1, 1], F32, tag="bar_out")
            ag_in = dramp.tile([128, HALF], F32, tag="ag_in")
            ag_out = dramp.tile([n_cores, 128, HALF], F32, tag="ag_out")
            ar1_in = dramp.tile([C, 2], F32, tag="ar1_in")
            ar1_out = dramp.tile([C, 2], F32, tag="ar1_out")
            ar2_in = dramp.tile([C, 2], F32, tag="ar2_in")
            ar2_out = dramp.tile([C, 2], F32, tag="ar2_out")

            # early dummy collective absorbs cross-core start skew
            nc.vector.memset(bar_sb[:], 0)
            nc.sync.dma_start(bar_in[:], bar_sb[:])
            nc.gpsimd.collective_compute(
                "AllReduce", ALU.add, replica_groups=rg,
                ins=[bar_in.opt()], outs=[bar_out.opt()])

            # ---------------- constants / weights prep ----------------
            nc.gpsimd.memset(xpad[:], 0)
            nc.gpsimd.memset(ypad[:], 0)
            nc.gpsimd.memset(yst[:], 0)
            nc.vector.memset(stats[:], 0)
            nc.vector.memset(q2s[:], 0)
            nc.vector.memset(onesKM[:], 1.0)
            nc.vector.memset(onesv[:], 1.0)
            nc.vector.memset(eps_t[:], EPS)
            nc.vector.memset(lh[:, 0:1], 0.0)
            nc.vector.memset(lh[:, 1:2], 1.0)

            lhsw = pp.tile([128, 2, 2, 3, 128], BF16, tag="lhsw")
            nc.sync.dma_start(lhsw[:], lhsw_in[:])
            lhs = [[[lhsw[:, ci, par, dy, :] for dy in range(3)]
                    for par in range(2)] for ci in range(2)]

            nc.sync.dma_start(fc1T[:], fc1t_in[:])
            nc.sync.dma_start(fc2T[:], fc2t_in[:])
            nc.sync.dma_start(fc1b[:], fc1bp_in[:])
            fc2b = pp.tile([128, 1], F32, tag="fc2b")
            nc.sync.dma_start(fc2b[:], fc2bp_in[:])
            nc.sync.dma_start(vecs[:, 0:4], vecs_in[:])

            # ---------------- group table ----------------
            # group gi = 2j + par ; par 0 = L (parts 0:64), 1 = H (64:128)
            groups = []
            for (j, s0, ns) in pairs:
                for par in range(2):
                    groups.append((j, par, s0, ns))

            def resid_load(gidx):
                (j, par, s0, ns) = groups[gidx]
                h = 64 * par
                nc.scalar.dma_start(
                    resid[h:h + 64, s0 * HW:(s0 + ns) * HW],
                    xr_in[:, HALF * par + s0:HALF * par + s0 + ns]
                    .rearrange("p b h w -> p (b h w)"))

            # ---------------- deferred-work schedule ----------------
            pool_at = {}      # group -> list of pair js
            for j in range(NP):
                pool_at.setdefault(3 + j // 2, []).append(j)
            FC_AT = 3 + (NP - 1) // 2 + 1          # 18
            bis_at = {}
            g = FC_AT + 2
            for _ in range(BISECT_ITERS):
                bis_at[g] = 1
                g += 1
            SEP_AT = g             # threshold ready; make sep
            p3a_at = {}
            nxt = SEP_AT + 1
            for j in range(NP):
                gg = max(nxt, 2 * j + 2)
                if gg < NG:
                    p3a_at.setdefault(gg, []).append(j)
                    nxt = gg + 1
                # else: handled post-loop
            p3a_done = {j for v in p3a_at.values() for j in v}

            # ---------------- helper ops ----------------
            def conv_mms(ci, par, slot, ns, pad):
                ps = psc.tile([128, 3, 512], F32, tag="cps")
                tl = _tiles(ns)
                for dy in range(3):
                    for (t, (tb0, nb)) in enumerate(tl):
                        nc.tensor.matmul(
                            ps[:, t, 0:nb * 72].rearrange(
                                "p (b r w) -> p b r w", b=nb, r=8, w=9),
                            lhs[ci][par][dy],
                            pad[:, slot, tb0:tb0 + nb, dy:dy + 8, 1:10],
                            start=(dy == 0), stop=(dy == 2))
                return ps, tl

            def evac(ci, gi, par, s0, ns, ps, tl):
                """A->R, B->cmb, gpsimd dma-add cmb into R."""
                h = 64 * par
                ho = 64 - h
                c0 = s0 * HW
                W = ns * HW
                cmb = rp.tile([128, GB * HW], BF16, tag="cmb")
                a_on_act = (gi % 2 == 0)
                full = (ns == GB)
                if full:
                    srcA = ps[h:h + 64, :, 0:432].rearrange(
                        "p t (b r w) -> p t b r w", b=6, r=8, w=9)[
                        :, :, :, :, 0:8]
                    srcB = ps[ho:ho + 64, :, 0:432].rearrange(
                        "p t (b r w) -> p t b r w", b=6, r=8, w=9)[
                        :, :, :, :, 1:9]
                    dstA = R[h:h + 64, c0:c0 + W].rearrange(
                        "p (t b r w) -> p t b r w", t=3, b=6, r=8, w=8)
                    dstB = cmb[ho:ho + 64, 0:W].rearrange(
                        "p (t b r w) -> p t b r w", t=3, b=6, r=8, w=8)
                else:
                    # partial group: per-tile evacs
                    for (t, (tb0, nb)) in enumerate(tl):
                        sA = ps[h:h + 64, t, 0:nb * 72].rearrange(
                            "p (b r w) -> p b r w", b=nb, r=8, w=9)[
                            :, :, :, 0:8]
                        sB = ps[ho:ho + 64, t, 0:nb * 72].rearrange(
                            "p (b r w) -> p b r w", b=nb, r=8, w=9)[
                            :, :, :, 1:9]
                        dA = R[h:h + 64, c0 + tb0 * HW:
                               c0 + (tb0 + nb) * HW].rearrange(
                            "p (b r w) -> p b r w", b=nb, r=8, w=8)
                        dB = cmb[ho:ho + 64, tb0 * HW:
                                 (tb0 + nb) * HW].rearrange(
                            "p (b r w) -> p b r w", b=nb, r=8, w=8)
                        if ci == 0:
                            nc.scalar.activation(dA, sA, AF.Copy)
                            nc.vector.tensor_copy(dB, sB)
                        else:
                            cia = 122 + (gi - 56) if t == 1 else 64 + gi
                            cib = 186 + (gi - 56) if t == 1 else 128 + gi
                            nc.scalar.activation(
                                dA, sA, AF.Copy,
                                accum_out=stats[h:h + 64, cia:cia + 1])
                            nc.vector.tensor_scalar(
                                out=dB, in0=sB, scalar1=1.0, scalar2=0.0,
                                op0=ALU.mult, op1=ALU.add,
                                accum_out=stats[ho:ho + 64, cib:cib + 1])
                    nc.gpsimd.dma_start(R[h:h + 64, c0:c0 + W],
                                        cmb[ho:ho + 64, 0:W],
                                        accum_op=ALU.add)
                    return
                if ci == 0:
                    nc.scalar.activation(dstA, srcA, AF.Copy)
                    nc.vector.tensor_copy(dstB, srcB)
                else:
                    ca = stats[:, 64 + gi:65 + gi]
                    cb = stats[:, 128 + gi:129 + gi]
                    nc.scalar.activation(dstA, srcA, AF.Copy,
                                         accum_out=ca[h:h + 64, :])
                    nc.vector.tensor_scalar(
                        out=dstB, in0=srcB, scalar1=1.0, scalar2=0.0,
                        op0=ALU.mult, op1=ALU.add,
                        accum_out=cb[ho:ho + 64, :])
                nc.gpsimd.dma_start(R[h:h + 64, c0:c0 + W],
                                    cmb[ho:ho + 64, 0:W],
                                    accum_op=ALU.add)

            def bisect_iter():
                tj = smallp.tile([128, 2], F32, tag="bj")
                nc.vector.tensor_scalar(out=tj[:], in0=lh[:], scalar1=0.5,
                                        scalar2=None, op0=ALU.mult,
                                        op1=ALU.add, accum_out=Tt[:])
                cnt = smallp.tile([128, 1], F32, tag="bcnt")
                nc.vector.scalar_tensor_tensor(
                    out=cjunk[:], in0=gata_s[:].rearrange("p n b -> p (n b)"),
                    scalar=Tt[:], in1=onesv[:], op0=ALU.is_lt, op1=ALU.mult,
                    accum_out=cnt[:])
                cntb = smallp.tile([128, 1], BF16, tag="bcntb")
                nc.vector.tensor_copy(cntb[:], cnt[:])
                psum_c = psf.tile([128, 512], F32, tag="bps")
                nc.tensor.matmul(psum_c[:, 0:1], onesKM[:], cntb[:],
                                 start=True, stop=True)
                m_le = smallp.tile([128, 1], I32, tag="bmle")
                m_gt = smallp.tile([128, 1], I32, tag="bmgt")
                nc.vector.tensor_scalar(out=m_le[:], in0=psum_c[:, 0:1],
                                        scalar1=float(k_sub), scalar2=None,
                                        op0=ALU.is_le)
                nc.vector.tensor_scalar(out=m_gt[:], in0=psum_c[:, 0:1],
                                        scalar1=float(k_sub), scalar2=None,
                                        op0=ALU.is_gt)
                nc.vector.copy_predicated(out=lh[:, 0:1], mask=m_le[:],
                                          data=Tt[:])
                nc.vector.copy_predicated(out=lh[:, 1:2], mask=m_gt[:],
                                          data=Tt[:])

            def p3a_pair(j, s0, ns):
                c0 = s0 * HW
                W = ns * HW
                rv = R[:, c0:c0 + W].rearrange("p (b q) -> p b q", b=ns)
                sb = sep[:, s0:s0 + ns].unsqueeze(2).broadcast_to(
                    (128, ns, HW))
                nc.vector.scalar_tensor_tensor(
                    out=rv, in0=rv, scalar=1.0, in1=sb,
                    op0=ALU.mult, op1=ALU.mult,
                    accum_out=stats[:, j:j + 1])
                sqj = rp.tile([128, GB * HW], BF16, tag="sqj")
                nc.scalar.activation(
                    sqj[:, 0:W], R[:, c0:c0 + W], AF.Square,
                    accum_out=stats[:, 32 + j:33 + j])

            def pool_pair(s0, ns):
                pt = rp.tile([128, GB * 32], BF16, tag="ptmp")
                rv32 = resid[:, s0 * HW:(s0 + ns) * HW].rearrange(
                    "p (b h q) -> p b h q", b=ns, h=2, q=32)
                nc.gpsimd.tensor_tensor(
                    out=pt[:, 0:ns * 32].rearrange(
                        "p (b q) -> p b q", b=ns),
                    in0=rv32[:, :, 0, :], in1=rv32[:, :, 1, :], op=ALU.add)
                nc.vector.tensor_reduce(
                    out=pooled[:, s0:s0 + ns],
                    in_=pt[:, 0:ns * 32].rearrange("p (b q) -> p b q", b=ns),
                    axis=AX.X, op=ALU.add)

            def fc_chain():
                zp = psf.tile([128, 512], F32, tag="zfc")
                nc.tensor.matmul(zp[0:32, 0:HALF], fc1T[:],
                                 pooled[:, 0:HALF], start=True, stop=True)
                z1 = smallp.tile([32, 512], F32, tag="z1")
                nc.scalar.activation(z1[:, 0:HALF], zp[0:32, 0:HALF],
                                     AF.Relu, scale=1.0 / HW, bias=fc1b[:])
                zp2 = psf.tile([128, 512], F32, tag="zfc")
                nc.tensor.matmul(zp2[:, 0:HALF], fc2T[:],
                                 z1[:, 0:HALF], start=True, stop=True)
                nc.scalar.activation(gates[:, 0:HALF], zp2[:, 0:HALF],
                                     AF.Sigmoid, bias=fc2b[:])
                nc.sync.dma_start(ag_in[:], gates[:])
                nc.gpsimd.collective_compute(
                    "AllGather", ALU.bypass, replica_groups=rg,
                    ins=[ag_in.opt()], outs=[ag_out.opt()])
                nc.gpsimd.dma_start(
                    gata_s[:],
                    ag_out[:, :, 0:SUB].rearrange("n p b -> p n b"))

            def make_sep():
                tj = smallp.tile([128, 2], F32, tag="bj")
                nc.vector.tensor_scalar(out=tj[:], in0=lh[:], scalar1=0.5,
                                        scalar2=None, op0=ALU.mult,
                                        op1=ALU.add, accum_out=Tt[:])
                nc.vector.tensor_scalar(out=negT[:], in0=Tt[:], scalar1=-1.0,
                                        scalar2=None, op0=ALU.mult)
                nc.scalar.activation(sep[:], gates[:], AF.Relu,
                                     bias=negT[:])

            def deferred(gi):
                for j in pool_at.get(gi, []):
                    pool_pair(pairs[j][1], pairs[j][2])
                if gi == FC_AT:
                    fc_chain()
                for _ in range(bis_at.get(gi, 0)):
                    bisect_iter()
                if gi == SEP_AT:
                    make_sep()
                for j in p3a_at.get(gi, []):
                    p3a_pair(j, pairs[j][1], pairs[j][2])

            # ---------------- conv1 loop ----------------
            NLD = (NG + 3) // 4
            for (gi, (j, par, s0, ns)) in enumerate(groups):
                slot = gi % 3
                h = 64 * par
                c0 = s0 * HW
                if gi < NLD:
                    for q in range(4 * gi, min(4 * gi + 4, NG)):
                        resid_load(q)
                # flat padded load straight from HBM
                nc.sync.dma_start(
                    xpad_f[0:64, slot, 0:ns * PADSZ],
                    x_in[:, HALF * par + s0:HALF * par + s0 + ns]
                    .rearrange("p b r w -> p (b r w)"))
                # flat-shift duplicate
                nc.sync.dma_start(
                    xpad_f[64:128, slot, 0:ns * PADSZ - 1],
                    xpad_f[0:64, slot, 1:ns * PADSZ])
                ps, tl = conv_mms(0, par, slot, ns, xpad)
                evac(0, gi, par, s0, ns, ps, tl)
                deferred(gi)

            # leftover deferred work
            for j in range(NP):
                if j not in p3a_done:
                    p3a_pair(j, pairs[j][1], pairs[j][2])

            # ---------------- BN1 allreduce ----------------
            def stats_ar(scol, qcol, slen, qt, arin, arout, cf, gcol, bcol):
                nc.vector.tensor_reduce(
                    out=sqt[:, 0:1], in_=stats[:, scol:scol + slen],
                    axis=AX.X, op=ALU.add)
                if qt is None:
                    nc.vector.tensor_reduce(
                        out=sqt[:, 1:2], in_=stats[:, qcol:qcol + slen],
                        axis=AX.X, op=ALU.add)
                else:
                    nc.vector.tensor_reduce(
                        out=sqt[:, 1:2], in_=qt[:], axis=AX.X, op=ALU.add)
                nc.gpsimd.dma_start(sqt[0:64, 0:2], sqt[64:128, 0:2],
                                    accum_op=ALU.add)
                nc.sync.dma_start(arin[:], sqt[0:64, 0:2])
                nc.gpsimd.collective_compute(
                    "AllReduce", ALU.add, replica_groups=rg,
                    ins=[arin.opt()], outs=[arout.opt()])
                sq_g = smallp.tile([C, 2], F32, tag="sqg")
                nc.sync.dma_start(sq_g[:], arout[:])
                # scratch cols: 0=mean 1=E[x^2] 2=-var 3=sd 4=isd
                nc.vector.tensor_scalar(out=scratch[:, 0:2], in0=sq_g[:],
                                        scalar1=1.0 / N1, scalar2=None,
                                        op0=ALU.mult)
                nc.vector.scalar_tensor_tensor(
                    out=scratch[:, 2:3], in0=scratch[:, 0:1],
                    scalar=scratch[:, 0:1], in1=scratch[:, 1:2],
                    op0=ALU.mult, op1=ALU.subtract)
                nc.scalar.activation(scratch[:, 3:4], scratch[:, 2:3],
                                     AF.Sqrt, scale=-1.0, bias=eps_t[:])
                nc.vector.reciprocal(scratch[:, 4:5], scratch[:, 3:4])
                nc.vector.tensor_tensor(out=cf[0:64, 0:1],
                                        in0=vecs[0:64, gcol:gcol + 1],
                                        in1=scratch[:, 4:5], op=ALU.mult)
                nc.vector.scalar_tensor_tensor(
                    out=cf[0:64, 1:2], in0=scratch[:, 0:1],
                    scalar=cf[0:64, 0:1], in1=vecs[0:64, bcol:bcol + 1],
                    op0=ALU.mult, op1=ALU.subtract)
                nc.vector.tensor_scalar(out=cf[0:64, 1:2], in0=cf[0:64, 1:2],
                                        scalar1=-1.0, scalar2=None,
                                        op0=ALU.mult)
                nc.sync.dma_start(cf[64:128, :], cf[0:64, :])

            stats_ar(0, 32, 32, None, ar1_in, ar1_out, cf1, 0, 1)

            # ---------------- conv2 loop ----------------
            for (gi, (j, par, s0, ns)) in enumerate(groups):
                slot = gi % 3
                h = 64 * par
                c0 = s0 * HW
                rv = R[h:h + 64, c0:c0 + ns * HW].rearrange(
                    "p (b r w) -> p b r w", b=ns, r=8, w=8)
                if par == 0:
                    tb = rp.tile([64, GB * HW], BF16, tag="ytmp")
                    nc.vector.tensor_scalar(
                        out=tb[:, 0:ns * HW], in0=R[0:64, c0:c0 + ns * HW],
                        scalar1=cf1[0:64, 0:1], scalar2=cf1[0:64, 1:2],
                        op0=ALU.mult, op1=ALU.add)
                    nc.vector.tensor_scalar(
                        out=ypad[0:64, slot, 0:ns, 1:9, 2:10],
                        in0=tb[:, 0:ns * HW].rearrange(
                            "p (b r w) -> p b r w", b=ns, r=8, w=8),
                        scalar1=0.0, scalar2=None, op0=ALU.max)
                else:
                    ys = (gi // 2) % 2
                    nc.scalar.activation(
                        yst[64:128, ys, 0:ns, 1:9, 2:10], rv,
                        AF.Relu, scale=cf1[64:128, 0:1],
                        bias=cf1[64:128, 1:2])
                    nc.sync.dma_start(
                        ypad_f[0:64, slot, 0:ns * PADSZ],
                        yst[:].rearrange("p s b r w -> p s (b r w)")
                        [64:128, ys, 0:ns * PADSZ])
                nc.sync.dma_start(
                    ypad_f[64:128, slot, 0:ns * PADSZ - 1],
                    ypad_f[0:64, slot, 1:ns * PADSZ])
                ps, tl = conv_mms(1, par, slot, ns, ypad)
                evac(1, gi, par, s0, ns, ps, tl)
                if par == 1:
                    # Q2 over the completed pair
                    c0p = pairs[j][1] * HW
                    Wp = pairs[j][2] * HW
                    sqj = rp.tile([128, GB * HW], BF16, tag="sqj")
                    if j % 2 == 0:
                        nc.vector.scalar_tensor_tensor(
                            out=sqj[:, 0:Wp], in0=R[:, c0p:c0p + Wp],
                            scalar=1.0, in1=R[:, c0p:c0p + Wp],
                            op0=ALU.mult, op1=ALU.mult,
                            accum_out=q2s[:, j:j + 1])
                    else:
                        nc.scalar.activation(
                            sqj[:, 0:Wp], R[:, c0p:c0p + Wp], AF.Square,
                            accum_out=q2s[:, j:j + 1])

            stats_ar(64, 0, 124, q2s, ar2_in, ar2_out, cf2, 2, 3)

            # ---------------- P5 ----------------
            p5p_cm = tc.tile_pool(name="p5p", bufs=2)
            p5p = p5p_cm.__enter__()
            p5o_cm = tc.tile_pool(name="p5o", bufs=2)
            p5o = p5o_cm.__enter__()
            for (j, s0, ns) in pairs:
                c0 = s0 * HW
                W = ns * HW
                tmp = p5p.tile([128, GB * HW], BF16, tag="p5t")
                obuf = p5o.tile([128, GB * HW], BF16, tag="obuf")
                nc.vector.tensor_scalar(
                    out=tmp[:, 0:W], in0=R[:, c0:c0 + W],
                    scalar1=cf2[:, 0:1], scalar2=None, op0=ALU.mult)
                nc.vector.tensor_tensor(
                    out=tmp[:, 0:W], in0=tmp[:, 0:W],
                    in1=resid[:, c0:c0 + W], op=ALU.add)
                nc.scalar.activation(
                    obuf[:, 0:W], tmp[:, 0:W], AF.Relu, bias=cf2[:, 1:2])
                nc.sync.dma_start(
                    out_d[:, s0:s0 + ns],
                    obuf[0:64, 0:W].rearrange("p (b h w) -> p b h w",
                                              b=ns, h=8, w=8))
                nc.scalar.dma_start(
                    out_d[:, HALF + s0:HALF + s0 + ns],
                    obuf[64:128, 0:W].rearrange("p (b h w) -> p b h w",
                                                b=ns, h=8, w=8))
            p5o_cm.__exit__(None, None, None)
            p5p_cm.__exit__(None, None, None)

    nc.compile()
    return nc


_NC_CACHE = {}


def _get_nc(n_cores, b_loc):
    key = (n_cores, b_loc)
    if key not in _NC_CACHE:
        _NC_CACHE[key] = build_nc(n_cores, b_loc)
    return _NC_CACHE[key]


def make_in_maps(inputs, n_cores=8):
    import ml_dtypes

    x = np.asarray(inputs["x"], dtype=np.float32)
    b_loc = x.shape[0] // n_cores

    # block-packed conv lhs: [ci, par, dy, 128, 128] -> [128, ci, par, dy, 128]
    lhsw = np.zeros((2, 2, 3, 128, 128), dtype=np.float32)
    for ci, w in ((0, inputs["conv1_w"]), (1, inputs["conv2_w"])):
        w = np.asarray(w, dtype=np.float32)
        for par in range(2):
            ma, mb = (0, 64) if par == 0 else (64, 0)
            for dy in range(3):
                lhsw[ci, par, dy, 0:64, ma:ma + 64] = w[:, :, dy, 0].T
                lhsw[ci, par, dy, 64:128, ma:ma + 64] = w[:, :, dy, 1].T
                lhsw[ci, par, dy, 64:128, mb:mb + 64] = w[:, :, dy, 2].T
    lhsw = np.ascontiguousarray(lhsw.transpose(3, 0, 1, 2, 4)).astype(
        ml_dtypes.bfloat16)

    f1 = np.asarray(inputs["fc1_w"], dtype=np.float32)    # [16, 64]
    f2 = np.asarray(inputs["fc2_w"], dtype=np.float32)    # [64, 16]
    fc1t = np.zeros((128, 32), dtype=np.float32)
    fc1t[0:64, 0:16] = f1.T
    fc1t[64:128, 16:32] = f1.T
    fc2t = np.zeros((32, 128), dtype=np.float32)
    fc2t[0:16, 0:64] = f2.T
    fc2t[16:32, 64:128] = f2.T
    fc1bp = np.tile(np.asarray(inputs["fc1_b"], np.float32), 2)[:, None]
    fc2bp = np.tile(np.asarray(inputs["fc2_b"], np.float32), 2)[:, None]
    vecsp = np.stack([np.tile(np.asarray(inputs[k], np.float32), 2)
                      for k in ("bn1_g", "bn1_b", "bn2_g", "bn2_b")], axis=1)
    wm = {"lhsw": lhsw, "fc1t": np.ascontiguousarray(fc1t),
          "fc2t": np.ascontiguousarray(fc2t), "fc1bp": fc1bp,
          "fc2bp": fc2bp, "vecsp": np.ascontiguousarray(vecsp)}

    in_maps = []
    for c in range(n_cores):
        xc = x[c * b_loc:(c + 1) * b_loc].transpose(1, 0, 2, 3)
        xr = np.ascontiguousarray(xc).astype(ml_dtypes.bfloat16)
        xp = np.zeros((64, b_loc, 10, 10), dtype=ml_dtypes.bfloat16)
        xp[:, :, 1:9, 2:10] = xr
        m = {"x": xp, "xr": xr}
        m.update(wm)
        in_maps.append(m)
    return in_maps


def kernel(**inputs):
    from concourse.bass_utils import run_bass_kernel_spmd

    x = np.asarray(inputs["x"], dtype=np.float32)
    B = x.shape[0]
    n_cores = 8
    b_loc = B // n_cores
    nc = _get_nc(n_cores, b_loc)
    in_maps = make_in_maps(inputs, n_cores)
    res = run_bass_kernel_spmd(nc, in_maps, core_ids=list(range(n_cores)))
    outs = []
    for c in range(n_cores):
        oc = np.asarray(res.results[c]["out"]).astype(np.float32)
        outs.append(oc.transpose(1, 0, 2, 3))
    return np.concatenate(outs, axis=0)


# revision 20
# speedup vs baseline: 1.0168x; 1.0168x over previous
"""Trainium2 Bass kernel for nn_BasicBlock (conv-SE-prune-BN residual block).

Data-parallel over batch across 8 NeuronCores; b_loc = 1024 per core.

v3 design (vs baseline): single x load, everything SBUF-resident.
 - Host pre-transposes x to [C, b_loc, 8, 8] and casts to bf16; output is
   returned bf16 [C, b_loc, 8, 8] and cast back on host.
 - Samples are split into two partition halves: batch 0:512 lives on
   partitions 0:64 ("L"), batch 512:1024 on 64:128 ("H"), giving 128-wide
   elementwise ops. Conv groups alternate L/H; the conv lhs has an L and
   an H variant (output accumulator halves swapped) so conv outputs land
   on their home partitions.
 - Conv: 3 matmuls per 6-sample tile, K=128 = channels + flat-shifted
   duplicate, M=128 = two accumulators (A, B). A evacuated by ACT/DVE
   copy psum->SBUF bf16; B evacuated to a bounce buffer and merged into
   R by a gpsimd DMA with accum_op=add (CCE inline add, crosses
   partitions for free).
 - Padded layout per sample is 10 rows x 10 cols, image at rows 1:9,
   cols 2:10 (so interior rows are 4-byte aligned for DVE 2x/4x modes).
   rhs views take cols 1:10; A-half out = view cols [0:8], B = [1:9].
 - Pooling from the resident bf16 copy (pair-packed, 128 partitions),
   fc gates computed pair-packed via block-packed fc weights, AllGather,
   then threshold bisection (14 iters) on a 1/8 subsample, all
   interleaved with conv1 groups. P3a (gate apply + BN1 stats) also
   interleaves with conv1's back half.
 - BN stats: per-group accum_out columns, reduced + partition-folded
   (gpsimd DMA add) + AllReduduced; coefs duplicated to both halves.
 - P5 (bn2 affine + residual + relu) runs pair-packed from SBUF.
"""
import numpy as np

import concourse.bacc as bacc
import concourse.bass as bass
import concourse.mybir as mybir
import concourse.tile as tile

F32 = mybir.dt.float32
BF16 = mybir.dt.bfloat16
I32 = mybir.dt.int32
AF = mybir.ActivationFunctionType
ALU = mybir.AluOpType
AX = mybir.AxisListType

C = 64
HW = 64
TILE_B = 6
GB = 18          # samples per conv group
PRUNE_RATE = 0.2
EPS = 1e-5
BISECT_ITERS = 12
SUB = 64         # bisect subsample columns per (core, partition)
PR, PW = 10, 10  # padded rows / cols per sample
PADSZ = PR * PW


def _pairs(half):
    """[(j, s0, ns)] covering one 512-sample half by 18-sample groups."""
    out = []
    s0 = 0
    j = 0
    while s0 < half:
        ns = min(GB, half - s0)
        out.append((j, s0, ns))
        s0 += ns
        j += 1
    return out


def _tiles(ns):
    t, b0 = [], 0
    while b0 < ns:
        nb = min(TILE_B, ns - b0)
        t.append((b0, nb))
        b0 += nb
    return t


def _transpose64(nc, dst_ap, src_ap):
    for i in (0, 32):
        for j in (0, 32):
            nc.vector.transpose(out=dst_ap[j:j + 32, i:i + 32],
                                in_=src_ap[i:i + 32, j:j + 32])


def build_nc(n_cores, b_loc):
    B_glob = n_cores * b_loc
    HALF = b_loc // 2
    N1 = float(B_glob * HW)
    n_sub = n_cores * 128 * SUB
    k_sub = PRUNE_RATE * n_sub
    D0s = float(2.0 * k_sub - n_sub)
    rg = [list(range(n_cores))]

    pairs = _pairs(HALF)          # 29 pairs
    NP = len(pairs)
    NG = 2 * NP                   # 58 groups, order L0 H0 L1 H1 ...

    nc = bacc.Bacc("TRN2", target_bir_lowering=False, debug=False,
                   enable_asserts=True, num_devices=n_cores)

    x_in = nc.dram_tensor("x", [C, b_loc, PR, PW], BF16,
                          kind="ExternalInput")
    xr_in = nc.dram_tensor("xr", [C, b_loc, 8, 8], BF16,
                           kind="ExternalInput")
    lhsw_in = nc.dram_tensor("lhsw", [128, 2, 2, 3, 128], BF16,
                             kind="ExternalInput")
    fc1t_in = nc.dram_tensor("fc1t", [128, 32], F32, kind="ExternalInput")
    fc2t_in = nc.dram_tensor("fc2t", [32, 128], F32, kind="ExternalInput")
    fc1bp_in = nc.dram_tensor("fc1bp", [32, 1], F32, kind="ExternalInput")
    fc2bp_in = nc.dram_tensor("fc2bp", [128, 1], F32, kind="ExternalInput")
    vecs_in = nc.dram_tensor("vecsp", [128, 4], F32, kind="ExternalInput")
    out_d = nc.dram_tensor("out", [C, b_loc, 8, 8], BF16, kind="ExternalOutput")

    with tile.TileContext(nc) as tc:
        with (
            tc.tile_pool(name="persist", bufs=1) as pp,
            tc.tile_pool(name="rings", bufs=2) as rp,
            tc.tile_pool(name="small", bufs=2) as smallp,
            tc.tile_pool(name="psc", bufs=2, space="PSUM") as psc,
            tc.tile_pool(name="psf", bufs=1, space="PSUM") as psf,
            tc.tile_pool(name="dram", bufs=1, space="DRAM") as dramp,
        ):
            # ---------------- persistent SBUF ----------------
            resid = pp.tile([128, HALF * HW], BF16, tag="resid")
            R = pp.tile([128, HALF * HW], BF16, tag="R")
            xpad = pp.tile([128, 3, GB, PR, PW], BF16, tag="xpad")
            ypad = pp.tile([128, 3, GB, PR, PW], BF16, tag="ypad")
            pooled = pp.tile([128, HALF], F32, tag="pooled")
            gates = pp.tile([128, HALF], F32, tag="gates")
            sep = pp.tile([128, HALF], BF16, tag="sep")
            gata_s = pp.tile([128, n_cores, SUB], BF16, tag="gata_s")
            onesKM = pp.tile([128, 128], BF16, tag="onesKM")
            stats = pp.tile([128, 192], F32, tag="stats")
            q2s = pp.tile([128, 32], F32, tag="q2s")
            vecs = pp.tile([128, 8], F32, tag="vecs")
            # vecs cols: 0=bn1_g 1=bn1_b 2=bn2_g 3=bn2_b
            fc1b = pp.tile([32, 1], F32, tag="fc1b")
            fc1T = pp.tile([128, 32], F32, tag="fc1T")
            fc2T = pp.tile([32, 128], F32, tag="fc2T")
            cf1 = pp.tile([128, 2], F32, tag="cf1")
            cf2 = pp.tile([128, 2], F32, tag="cf2")
            eps_t = pp.tile([C, 1], F32, tag="eps")
            lh = pp.tile([128, 2], F32, tag="lh")
            Tt = pp.tile([128, 1], F32, tag="Tt")
            negT = pp.tile([128, 1], F32, tag="negT")
            cjunk = pp.tile([128, n_cores * SUB], BF16, tag="cjunk")
            onesv = pp.tile([128, n_cores * SUB], BF16, tag="onesv")
            yst = pp.tile([128, 2, GB, PR, PW], BF16, tag="yst")
            scratch = pp.tile([C, 8], F32, tag="scratch")
            sqt = pp.tile([128, 4], F32, tag="sqt")

            xpad_f = xpad[:].rearrange("p s b r w -> p s (b r w)")
            ypad_f = ypad[:].rearrange("p s b r w -> p s (b r w)")

            # dram bounce buffers
            bar_sb = pp.tile([1, 1], F32, tag="bar_sb")
            bar_in = dramp.tile([1, 1], F32, tag="bar_in")
            bar_out = dramp.tile([1, 1], F32, tag="bar_out")
            ag_in = dramp.tile([128, HALF], F32, tag="ag_in")
            ag_out = dramp.tile([n_cores, 128, HALF], F32, tag="ag_out")
            ar1_in = dramp.tile([C, 2], F32, tag="ar1_in")
            ar1_out = dramp.tile([C, 2], F32, tag="ar1_out")
            ar2_in = dramp.tile([C, 2], F32, tag="ar2_in")
            ar2_out = dramp.tile([C, 2], F32, tag="ar2_out")

            # early dummy collective absorbs cross-core start skew
            nc.vector.memset(bar_sb[:], 0)
            nc.sync.dma_start(bar_in[:], bar_sb[:])
            nc.gpsimd.collective_compute(
                "AllReduce", ALU.add, replica_groups=rg,
                ins=[bar_in.opt()], outs=[bar_out.opt()])

            # ---------------- constants / weights prep ----------------
            nc.gpsimd.memset(xpad[:], 0)
            nc.gpsimd.memset(ypad[:], 0)
            nc.gpsimd.memset(yst[:], 0)
            nc.vector.memset(stats[:], 0)
            nc.vector.memset(q2s[:], 0)
            nc.vector.memset(onesKM[:], 1.0)
            nc.vector.memset(onesv[:], 1.0)
            nc.vector.memset(eps_t[:], EPS)
            nc.vector.memset(lh[:, 0:1], 0.0)
            nc.vector.memset(lh[:, 1:2], 1.0)

            lhsw = pp.tile([128, 2, 2, 3, 128], BF16, tag="lhsw")
            nc.sync.dma_start(lhsw[:], lhsw_in[:])
            lhs = [[[lhsw[:, ci, par, dy, :] for dy in range(3)]
                    for par in range(2)] for ci in range(2)]

            nc.sync.dma_start(fc1T[:], fc1t_in[:])
            nc.sync.dma_start(fc2T[:], fc2t_in[:])
            nc.sync.dma_start(fc1b[:], fc1bp_in[:])
            fc2b = pp.tile([128, 1], F32, tag="fc2b")
            nc.sync.dma_start(fc2b[:], fc2bp_in[:])
            nc.sync.dma_start(vecs[:, 0:4], vecs_in[:])

            # ---------------- group table ----------------
            # group gi = 2j + par ; par 0 = L (parts 0:64), 1 = H (64:128)
            groups = []
            for (j, s0, ns) in pairs:
                for par in range(2):
                    groups.append((j, par, s0, ns))

            def resid_load(gidx):
                (j, par, s0, ns) = groups[gidx]
                h = 64 * par
                nc.scalar.dma_start(
                    resid[h:h + 64, s0 * HW:(s0 + ns) * HW],
                    xr_in[:, HALF * par + s0:HALF * par + s0 + ns]
                    .rearrange("p b h w -> p (b h w)"))

            # ---------------- deferred-work schedule ----------------
            pool_at = {}      # group -> list of pair js
            for j in range(NP):
                pool_at.setdefault(3 + j // 2, []).append(j)
            FC_AT = 3 + (NP - 1) // 2 + 1          # 18
            bis_at = {}
            g = FC_AT + 2
            for _ in range(BISECT_ITERS):
                bis_at[g] = 1
                g += 1
            SEP_AT = g             # threshold ready; make sep
            p3a_at = {}
            nxt = SEP_AT + 1
            for j in range(NP):
                gg = max(nxt, 2 * j + 2)
                if gg < NG:
                    p3a_at.setdefault(gg, []).append(j)
                    nxt = gg + 1
                # else: handled post-loop
            p3a_done = {j for v in p3a_at.values() for j in v}

            # ---------------- helper ops ----------------
            def conv_mms(ci, par, slot, ns, pad):
                ps = psc.tile([128, 3, 512], F32, tag="cps")
                tl = _tiles(ns)
                for dy in range(3):
                    for (t, (tb0, nb)) in enumerate(tl):
                        nc.tensor.matmul(
                            ps[:, t, 0:nb * 72].rearrange(
                                "p (b r w) -> p b r w", b=nb, r=8, w=9),
                            lhs[ci][par][dy],
                            pad[:, slot, tb0:tb0 + nb, dy:dy + 8, 1:10],
                            start=(dy == 0), stop=(dy == 2))
                return ps, tl

            def evac(ci, gi, par, s0, ns, ps, tl):
                """A->R, B->cmb, gpsimd dma-add cmb into R."""
                h = 64 * par
                ho = 64 - h
                c0 = s0 * HW
                W = ns * HW
                cmb = rp.tile([128, GB * HW], BF16, tag="cmb")
                a_on_act = (gi % 2 == 0)
                full = (ns == GB)
                if full:
                    srcA = ps[h:h + 64, :, 0:432].rearrange(
                        "p t (b r w) -> p t b r w", b=6, r=8, w=9)[
                        :, :, :, :, 0:8]
                    srcB = ps[ho:ho + 64, :, 0:432].rearrange(
                        "p t (b r w) -> p t b r w", b=6, r=8, w=9)[
                        :, :, :, :, 1:9]
                    dstA = R[h:h + 64, c0:c0 + W].rearrange(
                        "p (t b r w) -> p t b r w", t=3, b=6, r=8, w=8)
                    dstB = cmb[ho:ho + 64, 0:W].rearrange(
                        "p (t b r w) -> p t b r w", t=3, b=6, r=8, w=8)
                else:
                    # partial group: per-tile evacs
                    for (t, (tb0, nb)) in enumerate(tl):
                        sA = ps[h:h + 64, t, 0:nb * 72].rearrange(
                            "p (b r w) -> p b r w", b=nb, r=8, w=9)[
                            :, :, :, 0:8]
                        sB = ps[ho:ho + 64, t, 0:nb * 72].rearrange(
                            "p (b r w) -> p b r w", b=nb, r=8, w=9)[
                            :, :, :, 1:9]
                        dA = R[h:h + 64, c0 + tb0 * HW:
                               c0 + (tb0 + nb) * HW].rearrange(
                            "p (b r w) -> p b r w", b=nb, r=8, w=8)
                        dB = cmb[ho:ho + 64, tb0 * HW:
                                 (tb0 + nb) * HW].rearrange(
                            "p (b r w) -> p b r w", b=nb, r=8, w=8)
                        if ci == 0:
                            nc.scalar.activation(dA, sA, AF.Copy)
                            nc.vector.tensor_copy(dB, sB)
                        else:
                            cia = 122 + (gi - 56) if t == 1 else 64 + gi
                            cib = 186 + (gi - 56) if t == 1 else 128 + gi
                            nc.scalar.activation(
                                dA, sA, AF.Copy,
                                accum_out=stats[h:h + 64, cia:cia + 1])
                            nc.vector.tensor_scalar(
                                out=dB, in0=sB, scalar1=1.0, scalar2=0.0,
                                op0=ALU.mult, op1=ALU.add,
                                accum_out=stats[ho:ho + 64, cib:cib + 1])
                    nc.gpsimd.dma_start(R[h:h + 64, c0:c0 + W],
                                        cmb[ho:ho + 64, 0:W],
                                        accum_op=ALU.add)
                    return
                if ci == 0:
                    nc.scalar.activation(dstA, srcA, AF.Copy)
                    nc.vector.tensor_copy(dstB, srcB)
                else:
                    ca = stats[:, 64 + gi:65 + gi]
                    cb = stats[:, 128 + gi:129 + gi]
                    nc.scalar.activation(dstA, srcA, AF.Copy,
                                         accum_out=ca[h:h + 64, :])
                    nc.vector.tensor_scalar(
                        out=dstB, in0=srcB, scalar1=1.0, scalar2=0.0,
                        op0=ALU.mult, op1=ALU.add,
                        accum_out=cb[ho:ho + 64, :])
                nc.gpsimd.dma_start(R[h:h + 64, c0:c0 + W],
                                    cmb[ho:ho + 64, 0:W],
                                    accum_op=ALU.add)

            def bisect_iter():
                tj = smallp.tile([128, 2], F32, tag="bj")
                nc.vector.tensor_scalar(out=tj[:], in0=lh[:], scalar1=0.5,
                                        scalar2=None, op0=ALU.mult,
                                        op1=ALU.add, accum_out=Tt[:])
                cnt = smallp.tile([128, 1], F32, tag="bcnt")
                nc.vector.scalar_tensor_tensor(
                    out=cjunk[:], in0=gata_s[:].rearrange("p n b -> p (n b)"),
                    scalar=Tt[:], in1=onesv[:], op0=ALU.is_lt, op1=ALU.mult,
                    accum_out=cnt[:])
                cntb = smallp.tile([128, 1], BF16, tag="bcntb")
                nc.vector.tensor_copy(cntb[:], cnt[:])
                psum_c = psf.tile([128, 512], F32, tag="bps")
                nc.tensor.matmul(psum_c[:, 0:1], onesKM[:], cntb[:],
                                 start=True, stop=True)
                m_le = smallp.tile([128, 1], I32, tag="bmle")
                m_gt = smallp.tile([128, 1], I32, tag="bmgt")
                nc.vector.tensor_scalar(out=m_le[:], in0=psum_c[:, 0:1],
                                        scalar1=float(k_sub), scalar2=None,
                                        op0=ALU.is_le)
                nc.vector.tensor_scalar(out=m_gt[:], in0=psum_c[:, 0:1],
                                        scalar1=float(k_sub), scalar2=None,
                                        op0=ALU.is_gt)
                nc.vector.copy_predicated(out=lh[:, 0:1], mask=m_le[:],
                                          data=Tt[:])
                nc.vector.copy_predicated(out=lh[:, 1:2], mask=m_gt[:],
                                          data=Tt[:])

            def p3a_pair(j, s0, ns):
                c0 = s0 * HW
                W = ns * HW
                rv = R[:, c0:c0 + W].rearrange("p (b q) -> p b q", b=ns)
                sb = sep[:, s0:s0 + ns].unsqueeze(2).broadcast_to(
                    (128, ns, HW))
                nc.vector.scalar_tensor_tensor(
                    out=rv, in0=rv, scalar=1.0, in1=sb,
                    op0=ALU.mult, op1=ALU.mult,
                    accum_out=stats[:, j:j + 1])
                sqj = rp.tile([128, GB * HW], BF16, tag="sqj")
                nc.scalar.activation(
                    sqj[:, 0:W], R[:, c0:c0 + W], AF.Square,
                    accum_out=stats[:, 32 + j:33 + j])

            def pool_pair(s0, ns):
                nc.vector.tensor_reduce(
                    out=pooled[:, s0:s0 + ns],
                    in_=resid[:, s0 * HW:(s0 + ns) * HW].rearrange(
                        "p (b q) -> p b q", b=ns),
                    axis=AX.X, op=ALU.add)

            def fc_chain():
                zp = psf.tile([128, 512], F32, tag="zfc")
                nc.tensor.matmul(zp[0:32, 0:HALF], fc1T[:],
                                 pooled[:, 0:HALF], start=True, stop=True)
                z1 = smallp.tile([32, 512], F32, tag="z1")
                nc.scalar.activation(z1[:, 0:HALF], zp[0:32, 0:HALF],
                                     AF.Relu, scale=1.0 / HW, bias=fc1b[:])
                zp2 = psf.tile([128, 512], F32, tag="zfc")
                nc.tensor.matmul(zp2[:, 0:HALF], fc2T[:],
                                 z1[:, 0:HALF], start=True, stop=True)
                nc.scalar.activation(gates[:, 0:HALF], zp2[:, 0:HALF],
                                     AF.Sigmoid, bias=fc2b[:])
                nc.sync.dma_start(ag_in[:], gates[:])
                nc.gpsimd.collective_compute(
                    "AllGather", ALU.bypass, replica_groups=rg,
                    ins=[ag_in.opt()], outs=[ag_out.opt()])
                nc.gpsimd.dma_start(
                    gata_s[:],
                    ag_out[:, :, 0:SUB].rearrange("n p b -> p n b"))

            def make_sep():
                tj = smallp.tile([128, 2], F32, tag="bj")
                nc.vector.tensor_scalar(out=tj[:], in0=lh[:], scalar1=0.5,
                                        scalar2=None, op0=ALU.mult,
                                        op1=ALU.add, accum_out=Tt[:])
                nc.vector.tensor_scalar(out=negT[:], in0=Tt[:], scalar1=-1.0,
                                        scalar2=None, op0=ALU.mult)
                nc.scalar.activation(sep[:], gates[:], AF.Relu,
                                     bias=negT[:])

            def deferred(gi):
                for j in pool_at.get(gi, []):
                    pool_pair(pairs[j][1], pairs[j][2])
                if gi == FC_AT:
                    fc_chain()
                for _ in range(bis_at.get(gi, 0)):
                    bisect_iter()
                if gi == SEP_AT:
                    make_sep()
                for j in p3a_at.get(gi, []):
                    p3a_pair(j, pairs[j][1], pairs[j][2])

            # ---------------- conv1 loop ----------------
            NLD = (NG + 3) // 4
            for (gi, (j, par, s0, ns)) in enumerate(groups):
                slot = gi % 3
                h = 64 * par
                c0 = s0 * HW
                if gi < NLD:
                    for q in range(4 * gi, min(4 * gi + 4, NG)):
                        resid_load(q)
                # flat padded load straight from HBM
                nc.sync.dma_start(
                    xpad_f[0:64, slot, 0:ns * PADSZ],
                    x_in[:, HALF * par + s0:HALF * par + s0 + ns]
                    .rearrange("p b r w -> p (b r w)"))
                # flat-shift duplicate
                nc.sync.dma_start(
                    xpad_f[64:128, slot, 0:ns * PADSZ - 1],
                    xpad_f[0:64, slot, 1:ns * PADSZ])
                ps, tl = conv_mms(0, par, slot, ns, xpad)
                evac(0, gi, par, s0, ns, ps, tl)
                deferred(gi)

            # leftover deferred work
            for j in range(NP):
                if j not in p3a_done:
                    p3a_pair(j, pairs[j][1], pairs[j][2])

            # ---------------- BN1 allreduce ----------------
            def stats_ar(scol, qcol, slen, qt, arin, arout, cf, gcol, bcol):
                nc.vector.tensor_reduce(
                    out=sqt[:, 0:1], in_=stats[:, scol:scol + slen],
                    axis=AX.X, op=ALU.add)
                if qt is None:
                    nc.vector.tensor_reduce(
                        out=sqt[:, 1:2], in_=stats[:, qcol:qcol + slen],
                        axis=AX.X, op=ALU.add)
                else:
                    nc.vector.tensor_reduce(
                        out=sqt[:, 1:2], in_=qt[:], axis=AX.X, op=ALU.add)
                nc.gpsimd.dma_start(sqt[0:64, 0:2], sqt[64:128, 0:2],
                                    accum_op=ALU.add)
                nc.sync.dma_start(arin[:], sqt[0:64, 0:2])
                nc.gpsimd.collective_compute(
                    "AllReduce", ALU.add, replica_groups=rg,
                    ins=[arin.opt()], outs=[arout.opt()])
                sq_g = smallp.tile([C, 2], F32, tag="sqg")
                nc.sync.dma_start(sq_g[:], arout[:])
                # scratch cols: 0=mean 1=E[x^2] 2=-var 3=sd 4=isd
                nc.vector.tensor_scalar(out=scratch[:, 0:2], in0=sq_g[:],
                                        scalar1=1.0 / N1, scalar2=None,
                                        op0=ALU.mult)
                nc.vector.scalar_tensor_tensor(
                    out=scratch[:, 2:3], in0=scratch[:, 0:1],
                    scalar=scratch[:, 0:1], in1=scratch[:, 1:2],
                    op0=ALU.mult, op1=ALU.subtract)
                nc.scalar.activation(scratch[:, 3:4], scratch[:, 2:3],
                                     AF.Sqrt, scale=-1.0, bias=eps_t[:])
                nc.vector.reciprocal(scratch[:, 4:5], scratch[:, 3:4])
                nc.vector.tensor_tensor(out=cf[0:64, 0:1],
                                        in0=vecs[0:64, gcol:gcol + 1],
                                        in1=scratch[:, 4:5], op=ALU.mult)
                nc.vector.scalar_tensor_tensor(
                    out=cf[0:64, 1:2], in0=scratch[:, 0:1],
                    scalar=cf[0:64, 0:1], in1=vecs[0:64, bcol:bcol + 1],
                    op0=ALU.mult, op1=ALU.subtract)
                nc.vector.tensor_scalar(out=cf[0:64, 1:2], in0=cf[0:64, 1:2],
                                        scalar1=-1.0, scalar2=None,
                                        op0=ALU.mult)
                nc.sync.dma_start(cf[64:128, :], cf[0:64, :])

            stats_ar(0, 32, 32, None, ar1_in, ar1_out, cf1, 0, 1)

            # ---------------- conv2 loop ----------------
            for (gi, (j, par, s0, ns)) in enumerate(groups):
                slot = gi % 3
                h = 64 * par
                c0 = s0 * HW
                rv = R[h:h + 64, c0:c0 + ns * HW].rearrange(
                    "p (b r w) -> p b r w", b=ns, r=8, w=8)
                if par == 0:
                    tb = rp.tile([64, GB * HW], BF16, tag="ytmp")
                    nc.vector.tensor_scalar(
                        out=tb[:, 0:ns * HW], in0=R[0:64, c0:c0 + ns * HW],
                        scalar1=cf1[0:64, 0:1], scalar2=cf1[0:64, 1:2],
                        op0=ALU.mult, op1=ALU.add)
                    nc.vector.tensor_scalar(
                        out=ypad[0:64, slot, 0:ns, 1:9, 2:10],
                        in0=tb[:, 0:ns * HW].rearrange(
                            "p (b r w) -> p b r w", b=ns, r=8, w=8),
                        scalar1=0.0, scalar2=None, op0=ALU.max)
                else:
                    ys = (gi // 2) % 2
                    nc.scalar.activation(
                        yst[64:128, ys, 0:ns, 1:9, 2:10], rv,
                        AF.Relu, scale=cf1[64:128, 0:1],
                        bias=cf1[64:128, 1:2])
                    nc.sync.dma_start(
                        ypad_f[0:64, slot, 0:ns * PADSZ],
                        yst[:].rearrange("p s b r w -> p s (b r w)")
                        [64:128, ys, 0:ns * PADSZ])
                nc.sync.dma_start(
                    ypad_f[64:128, slot, 0:ns * PADSZ - 1],
                    ypad_f[0:64, slot, 1:ns * PADSZ])
                ps, tl = conv_mms(1, par, slot, ns, ypad)
                evac(1, gi, par, s0, ns, ps, tl)
                if par == 1:
                    # Q2 over the completed pair
                    c0p = pairs[j][1] * HW
                    Wp = pairs[j][2] * HW
                    sqj = rp.tile([128, GB * HW], BF16, tag="sqj")
                    if j % 2 == 0:
                        nc.vector.scalar_tensor_tensor(
                            out=sqj[:, 0:Wp], in0=R[:, c0p:c0p + Wp],
                            scalar=1.0, in1=R[:, c0p:c0p + Wp],
                            op0=ALU.mult, op1=ALU.mult,
                            accum_out=q2s[:, j:j + 1])
                    else:
                        nc.scalar.activation(
                            sqj[:, 0:Wp], R[:, c0p:c0p + Wp], AF.Square,
                            accum_out=q2s[:, j:j + 1])

            stats_ar(64, 0, 124, q2s, ar2_in, ar2_out, cf2, 2, 3)

            # ---------------- P5 ----------------
            p5p_cm = tc.tile_pool(name="p5p", bufs=3)
            p5p = p5p_cm.__enter__()
            p5o_cm = tc.tile_pool(name="p5o", bufs=3)
            p5o = p5o_cm.__enter__()
            for (j, s0, ns) in pairs:
                c0 = s0 * HW
                W = ns * HW
                tmp = p5p.tile([128, GB * HW], BF16, tag="p5t")
                obuf = p5o.tile([128, GB * HW], BF16, tag="obuf")
                nc.vector.tensor_scalar(
                    out=tmp[:, 0:W], in0=R[:, c0:c0 + W],
                    scalar1=cf2[:, 0:1], scalar2=None, op0=ALU.mult)
                nc.vector.tensor_tensor(
                    out=tmp[:, 0:W], in0=tmp[:, 0:W],
                    in1=resid[:, c0:c0 + W], op=ALU.add)
                if j % 2 == 0:
                    nc.scalar.activation(
                        obuf[:, 0:W], tmp[:, 0:W], AF.Relu, bias=cf2[:, 1:2])
                else:
                    nc.vector.tensor_scalar(
                        out=obuf[:, 0:W], in0=tmp[:, 0:W],
                        scalar1=cf2[:, 1:2], scalar2=0.0,
                        op0=ALU.add, op1=ALU.max)
                nc.sync.dma_start(
                    out_d[:, s0:s0 + ns],
                    obuf[0:64, 0:W].rearrange("p (b h w) -> p b h w",
                                              b=ns, h=8, w=8))
                nc.scalar.dma_start(
                    out_d[:, HALF + s0:HALF + s0 + ns],
                    obuf[64:128, 0:W].rearrange("p (b h w) -> p b h w",
                                                b=ns, h=8, w=8))
            p5o_cm.__exit__(None, None, None)
            p5p_cm.__exit__(None, None, None)

    nc.compile()
    return nc


_NC_CACHE = {}


def _get_nc(n_cores, b_loc):
    key = (n_cores, b_loc)
    if key not in _NC_CACHE:
        _NC_CACHE[key] = build_nc(n_cores, b_loc)
    return _NC_CACHE[key]


def make_in_maps(inputs, n_cores=8):
    import ml_dtypes

    x = np.asarray(inputs["x"], dtype=np.float32)
    b_loc = x.shape[0] // n_cores

    # block-packed conv lhs: [ci, par, dy, 128, 128] -> [128, ci, par, dy, 128]
    lhsw = np.zeros((2, 2, 3, 128, 128), dtype=np.float32)
    for ci, w in ((0, inputs["conv1_w"]), (1, inputs["conv2_w"])):
        w = np.asarray(w, dtype=np.float32)
        for par in range(2):
            ma, mb = (0, 64) if par == 0 else (64, 0)
            for dy in range(3):
                lhsw[ci, par, dy, 0:64, ma:ma + 64] = w[:, :, dy, 0].T
                lhsw[ci, par, dy, 64:128, ma:ma + 64] = w[:, :, dy, 1].T
                lhsw[ci, par, dy, 64:128, mb:mb + 64] = w[:, :, dy, 2].T
    lhsw = np.ascontiguousarray(lhsw.transpose(3, 0, 1, 2, 4)).astype(
        ml_dtypes.bfloat16)

    f1 = np.asarray(inputs["fc1_w"], dtype=np.float32)    # [16, 64]
    f2 = np.asarray(inputs["fc2_w"], dtype=np.float32)    # [64, 16]
    fc1t = np.zeros((128, 32), dtype=np.float32)
    fc1t[0:64, 0:16] = f1.T
    fc1t[64:128, 16:32] = f1.T
    fc2t = np.zeros((32, 128), dtype=np.float32)
    fc2t[0:16, 0:64] = f2.T
    fc2t[16:32, 64:128] = f2.T
    fc1bp = np.tile(np.asarray(inputs["fc1_b"], np.float32), 2)[:, None]
    fc2bp = np.tile(np.asarray(inputs["fc2_b"], np.float32), 2)[:, None]
    vecsp = np.stack([np.tile(np.asarray(inputs[k], np.float32), 2)
                      for k in ("bn1_g", "bn1_b", "bn2_g", "bn2_b")], axis=1)
    wm = {"lhsw": lhsw, "fc1t": np.ascontiguousarray(fc1t),
          "fc2t": np.ascontiguousarray(fc2t), "fc1bp": fc1bp,
          "fc2bp": fc2bp, "vecsp": np.ascontiguousarray(vecsp)}

    in_maps = []
    for c in range(n_cores):
        xc = x[c * b_loc:(c + 1) * b_loc].transpose(1, 0, 2, 3)
        xr = np.ascontiguousarray(xc).astype(ml_dtypes.bfloat16)
        xp = np.zeros((64, b_loc, 10, 10), dtype=ml_dtypes.bfloat16)
        xp[:, :, 1:9, 2:10] = xr
        m = {"x": xp, "xr": xr}
        m.update(wm)
        in_maps.append(m)
    return in_maps


def kernel(**inputs):
    from concourse.bass_utils import run_bass_kernel_spmd

    x = np.asarray(inputs["x"], dtype=np.float32)
    B = x.shape[0]
    n_cores = 8
    b_loc = B // n_cores
    nc = _get_nc(n_cores, b_loc)
    in_maps = make_in_maps(inputs, n_cores)
    res = run_bass_kernel_spmd(nc, in_maps, core_ids=list(range(n_cores)))
    outs = []
    for c in range(n_cores):
        oc = np.asarray(res.results[c]["out"]).astype(np.float32)
        outs.append(oc.transpose(1, 0, 2, 3))
    return np.concatenate(outs, axis=0)


# revision 21
# speedup vs baseline: 1.0382x; 1.0211x over previous
"""Trainium2 Bass kernel for nn_BasicBlock (conv-SE-prune-BN residual block).

Data-parallel over batch across 8 NeuronCores; b_loc = 1024 per core.

v3 design (vs baseline): single x load, everything SBUF-resident.
 - Host pre-transposes x to [C, b_loc, 8, 8] and casts to bf16; output is
   returned bf16 [C, b_loc, 8, 8] and cast back on host.
 - Samples are split into two partition halves: batch 0:512 lives on
   partitions 0:64 ("L"), batch 512:1024 on 64:128 ("H"), giving 128-wide
   elementwise ops. Conv groups alternate L/H; the conv lhs has an L and
   an H variant (output accumulator halves swapped) so conv outputs land
   on their home partitions.
 - Conv: 3 matmuls per 6-sample tile, K=128 = channels + flat-shifted
   duplicate, M=128 = two accumulators (A, B). A evacuated by ACT/DVE
   copy psum->SBUF bf16; B evacuated to a bounce buffer and merged into
   R by a gpsimd DMA with accum_op=add (CCE inline add, crosses
   partitions for free).
 - Padded layout per sample is 10 rows x 10 cols, image at rows 1:9,
   cols 2:10 (so interior rows are 4-byte aligned for DVE 2x/4x modes).
   rhs views take cols 1:10; A-half out = view cols [0:8], B = [1:9].
 - Pooling from the resident bf16 copy (pair-packed, 128 partitions),
   fc gates computed pair-packed via block-packed fc weights, AllGather,
   then threshold bisection (14 iters) on a 1/8 subsample, all
   interleaved with conv1 groups. P3a (gate apply + BN1 stats) also
   interleaves with conv1's back half.
 - BN stats: per-group accum_out columns, reduced + partition-folded
   (gpsimd DMA add) + AllReduduced; coefs duplicated to both halves.
 - P5 (bn2 affine + residual + relu) runs pair-packed from SBUF.
"""
import numpy as np

import concourse.bacc as bacc
import concourse.bass as bass
import concourse.mybir as mybir
import concourse.tile as tile

F32 = mybir.dt.float32
BF16 = mybir.dt.bfloat16
I32 = mybir.dt.int32
AF = mybir.ActivationFunctionType
ALU = mybir.AluOpType
AX = mybir.AxisListType

C = 64
HW = 64
TILE_B = 6
GB = 18          # samples per conv group
PRUNE_RATE = 0.2
EPS = 1e-5
BISECT_ITERS = 10
SUB = 64         # bisect subsample columns per (core, partition)
PR, PW = 10, 10  # padded rows / cols per sample
PADSZ = PR * PW


def _pairs(half):
    """[(j, s0, ns)] covering one 512-sample half by 18-sample groups."""
    out = []
    s0 = 0
    j = 0
    while s0 < half:
        ns = min(GB, half - s0)
        out.append((j, s0, ns))
        s0 += ns
        j += 1
    return out


def _tiles(ns):
    t, b0 = [], 0
    while b0 < ns:
        nb = min(TILE_B, ns - b0)
        t.append((b0, nb))
        b0 += nb
    return t


def _transpose64(nc, dst_ap, src_ap):
    for i in (0, 32):
        for j in (0, 32):
            nc.vector.transpose(out=dst_ap[j:j + 32, i:i + 32],
                                in_=src_ap[i:i + 32, j:j + 32])


def build_nc(n_cores, b_loc):
    B_glob = n_cores * b_loc
    HALF = b_loc // 2
    N1 = float(B_glob * HW)
    n_sub = n_cores * 128 * SUB
    k_sub = PRUNE_RATE * n_sub
    D0s = float(2.0 * k_sub - n_sub)
    rg = [list(range(n_cores))]

    pairs = _pairs(HALF)          # 29 pairs
    NP = len(pairs)
    NG = 2 * NP                   # 58 groups, order L0 H0 L1 H1 ...

    nc = bacc.Bacc("TRN2", target_bir_lowering=False, debug=False,
                   enable_asserts=True, num_devices=n_cores)

    x_in = nc.dram_tensor("x", [C, b_loc, PR, PW], BF16,
                          kind="ExternalInput")
    xr_in = nc.dram_tensor("xr", [C, b_loc, 8, 8], BF16,
                           kind="ExternalInput")
    lhsw_in = nc.dram_tensor("lhsw", [128, 2, 2, 3, 128], BF16,
                             kind="ExternalInput")
    fc1t_in = nc.dram_tensor("fc1t", [128, 32], F32, kind="ExternalInput")
    fc2t_in = nc.dram_tensor("fc2t", [32, 128], F32, kind="ExternalInput")
    fc1bp_in = nc.dram_tensor("fc1bp", [32, 1], F32, kind="ExternalInput")
    fc2bp_in = nc.dram_tensor("fc2bp", [128, 1], F32, kind="ExternalInput")
    vecs_in = nc.dram_tensor("vecsp", [128, 4], F32, kind="ExternalInput")
    out_d = nc.dram_tensor("out", [C, b_loc, 8, 8], BF16, kind="ExternalOutput")

    with tile.TileContext(nc) as tc:
        with (
            tc.tile_pool(name="persist", bufs=1) as pp,
            tc.tile_pool(name="rings", bufs=2) as rp,
            tc.tile_pool(name="small", bufs=2) as smallp,
            tc.tile_pool(name="psc", bufs=2, space="PSUM") as psc,
            tc.tile_pool(name="psf", bufs=1, space="PSUM") as psf,
            tc.tile_pool(name="dram", bufs=1, space="DRAM") as dramp,
        ):
            # ---------------- persistent SBUF ----------------
            resid = pp.tile([128, HALF * HW], BF16, tag="resid")
            R = pp.tile([128, HALF * HW], BF16, tag="R")
            xpad = pp.tile([128, 3, GB, PR, PW], BF16, tag="xpad")
            ypad = pp.tile([128, 3, GB, PR, PW], BF16, tag="ypad")
            pooled = pp.tile([128, HALF], F32, tag="pooled")
            gates = pp.tile([128, HALF], F32, tag="gates")
            sep = pp.tile([128, HALF], BF16, tag="sep")
            gata_s = pp.tile([128, n_cores, SUB], F32, tag="gata_s")
            onesKM = pp.tile([128, 128], BF16, tag="onesKM")
            stats = pp.tile([128, 192], F32, tag="stats")
            q2s = pp.tile([128, 32], F32, tag="q2s")
            vecs = pp.tile([128, 8], F32, tag="vecs")
            # vecs cols: 0=bn1_g 1=bn1_b 2=bn2_g 3=bn2_b
            fc1b = pp.tile([32, 1], F32, tag="fc1b")
            fc1T = pp.tile([128, 32], F32, tag="fc1T")
            fc2T = pp.tile([32, 128], F32, tag="fc2T")
            cf1 = pp.tile([128, 2], F32, tag="cf1")
            cf2 = pp.tile([128, 2], F32, tag="cf2")
            eps_t = pp.tile([C, 1], F32, tag="eps")
            lh = pp.tile([128, 2], F32, tag="lh")
            Tt = pp.tile([128, 1], F32, tag="Tt")
            negT = pp.tile([128, 1], F32, tag="negT")
            cjunk = pp.tile([128, n_cores * SUB], F32, tag="cjunk")
            onesv = pp.tile([128, n_cores * SUB], F32, tag="onesv")
            yst = pp.tile([128, 2, GB, PR, PW], BF16, tag="yst")
            scratch = pp.tile([C, 8], F32, tag="scratch")
            sqt = pp.tile([128, 4], F32, tag="sqt")

            xpad_f = xpad[:].rearrange("p s b r w -> p s (b r w)")
            ypad_f = ypad[:].rearrange("p s b r w -> p s (b r w)")

            # dram bounce buffers
            bar_sb = pp.tile([1, 1], F32, tag="bar_sb")
            bar_in = dramp.tile([1, 1], F32, tag="bar_in")
            bar_out = dramp.tile([1, 1], F32, tag="bar_out")
            ag_in = dramp.tile([128, HALF], F32, tag="ag_in")
            ag_out = dramp.tile([n_cores, 128, HALF], F32, tag="ag_out")
            ar1_in = dramp.tile([C, 2], F32, tag="ar1_in")
            ar1_out = dramp.tile([C, 2], F32, tag="ar1_out")
            ar2_in = dramp.tile([C, 2], F32, tag="ar2_in")
            ar2_out = dramp.tile([C, 2], F32, tag="ar2_out")

            # xpad memset first so group-0 fill can start ASAP
            nc.gpsimd.memset(xpad[:], 0)
            # early dummy collective absorbs cross-core start skew
            nc.vector.memset(bar_sb[:], 0)
            nc.sync.dma_start(bar_in[:], bar_sb[:])
            nc.gpsimd.collective_compute(
                "AllReduce", ALU.add, replica_groups=rg,
                ins=[bar_in.opt()], outs=[bar_out.opt()])

            # ---------------- constants / weights prep ----------------
            nc.gpsimd.memset(ypad[:], 0)
            nc.gpsimd.memset(yst[:], 0)
            nc.vector.memset(stats[:], 0)
            nc.vector.memset(q2s[:], 0)
            nc.vector.memset(onesKM[:], 1.0)
            nc.vector.memset(onesv[:], 1.0)
            nc.vector.memset(eps_t[:], EPS)
            nc.vector.memset(lh[:, 0:1], 0.0)
            nc.vector.memset(lh[:, 1:2], 1.0)

            lhsw = pp.tile([128, 2, 2, 3, 128], BF16, tag="lhsw")
            nc.scalar.dma_start(lhsw[:], lhsw_in[:])
            lhs = [[[lhsw[:, ci, par, dy, :] for dy in range(3)]
                    for par in range(2)] for ci in range(2)]

            nc.scalar.dma_start(fc1T[:], fc1t_in[:])
            nc.scalar.dma_start(fc2T[:], fc2t_in[:])
            nc.scalar.dma_start(fc1b[:], fc1bp_in[:])
            fc2b = pp.tile([128, 1], F32, tag="fc2b")
            nc.scalar.dma_start(fc2b[:], fc2bp_in[:])
            nc.scalar.dma_start(vecs[:, 0:4], vecs_in[:])

            # ---------------- group table ----------------
            # group gi = 2j + par ; par 0 = L (parts 0:64), 1 = H (64:128)
            groups = []
            for (j, s0, ns) in pairs:
                for par in range(2):
                    groups.append((j, par, s0, ns))

            def resid_load(gidx):
                (j, par, s0, ns) = groups[gidx]
                h = 64 * par
                nc.scalar.dma_start(
                    resid[h:h + 64, s0 * HW:(s0 + ns) * HW],
                    xr_in[:, HALF * par + s0:HALF * par + s0 + ns]
                    .rearrange("p b h w -> p (b h w)"))

            # ---------------- deferred-work schedule ----------------
            pool_at = {}      # group -> list of pair js
            for j in range(NP):
                pool_at.setdefault(3 + j // 2, []).append(j)
            FC_AT = 3 + (NP - 1) // 2 + 1          # 18
            bis_at = {}
            g = FC_AT + 2
            for _ in range(BISECT_ITERS):
                bis_at[g] = 1
                g += 1
            SEP_AT = g             # threshold ready; make sep
            p3a_at = {}
            slot_used = {}
            for j in range(NP):
                gg = max(SEP_AT + 1 + j, 2 * j + 2)
                while gg < NG and slot_used.get(gg, 0) >= 2:
                    gg += 1
                # retry earlier second-slot if primary walk overflows
                if gg >= NG:
                    for g2 in range(max(2 * j + 2, SEP_AT + 1), NG):
                        if slot_used.get(g2, 0) < 2:
                            gg = g2
                            break
                if gg < NG:
                    p3a_at.setdefault(gg, []).append(j)
                    slot_used[gg] = slot_used.get(gg, 0) + 1
            p3a_done = {j for v in p3a_at.values() for j in v}

            # ---------------- helper ops ----------------
            def conv_mms(ci, par, slot, ns, pad):
                ps = psc.tile([128, 3, 512], F32, tag="cps")
                tl = _tiles(ns)
                for dy in range(3):
                    for (t, (tb0, nb)) in enumerate(tl):
                        nc.tensor.matmul(
                            ps[:, t, 0:nb * 72].rearrange(
                                "p (b r w) -> p b r w", b=nb, r=8, w=9),
                            lhs[ci][par][dy],
                            pad[:, slot, tb0:tb0 + nb, dy:dy + 8, 1:10],
                            start=(dy == 0), stop=(dy == 2))
                return ps, tl

            def evac(ci, gi, par, s0, ns, ps, tl):
                """A->R, B->cmb, gpsimd dma-add cmb into R."""
                h = 64 * par
                ho = 64 - h
                c0 = s0 * HW
                W = ns * HW
                cmb = rp.tile([128, GB * HW], BF16, tag="cmb")
                a_on_act = (gi % 2 == 0)
                full = (ns == GB)
                if full:
                    srcA = ps[h:h + 64, :, 0:432].rearrange(
                        "p t (b r w) -> p t b r w", b=6, r=8, w=9)[
                        :, :, :, :, 0:8]
                    srcB = ps[ho:ho + 64, :, 0:432].rearrange(
                        "p t (b r w) -> p t b r w", b=6, r=8, w=9)[
                        :, :, :, :, 1:9]
                    dstA = R[h:h + 64, c0:c0 + W].rearrange(
                        "p (t b r w) -> p t b r w", t=3, b=6, r=8, w=8)
                    dstB = cmb[ho:ho + 64, 0:W].rearrange(
                        "p (t b r w) -> p t b r w", t=3, b=6, r=8, w=8)
                else:
                    # partial group: per-tile evacs
                    for (t, (tb0, nb)) in enumerate(tl):
                        sA = ps[h:h + 64, t, 0:nb * 72].rearrange(
                            "p (b r w) -> p b r w", b=nb, r=8, w=9)[
                            :, :, :, 0:8]
                        sB = ps[ho:ho + 64, t, 0:nb * 72].rearrange(
                            "p (b r w) -> p b r w", b=nb, r=8, w=9)[
                            :, :, :, 1:9]
                        dA = R[h:h + 64, c0 + tb0 * HW:
                               c0 + (tb0 + nb) * HW].rearrange(
                            "p (b r w) -> p b r w", b=nb, r=8, w=8)
                        dB = cmb[ho:ho + 64, tb0 * HW:
                                 (tb0 + nb) * HW].rearrange(
                            "p (b r w) -> p b r w", b=nb, r=8, w=8)
                        if ci == 0:
                            nc.scalar.activation(dA, sA, AF.Copy)
                            nc.vector.tensor_copy(dB, sB)
                        else:
                            cia = 122 + (gi - 56) if t == 1 else 64 + gi
                            cib = 186 + (gi - 56) if t == 1 else 128 + gi
                            nc.scalar.activation(
                                dA, sA, AF.Copy,
                                accum_out=stats[h:h + 64, cia:cia + 1])
                            nc.vector.tensor_scalar(
                                out=dB, in0=sB, scalar1=1.0, scalar2=0.0,
                                op0=ALU.mult, op1=ALU.add,
                                accum_out=stats[ho:ho + 64, cib:cib + 1])
                    nc.gpsimd.dma_start(R[h:h + 64, c0:c0 + W],
                                        cmb[ho:ho + 64, 0:W],
                                        accum_op=ALU.add)
                    return
                if ci == 0:
                    nc.scalar.activation(dstA, srcA, AF.Copy)
                    nc.vector.tensor_copy(dstB, srcB)
                else:
                    ca = stats[:, 64 + gi:65 + gi]
                    cb = stats[:, 128 + gi:129 + gi]
                    nc.scalar.activation(dstA, srcA, AF.Copy,
                                         accum_out=ca[h:h + 64, :])
                    nc.vector.tensor_scalar(
                        out=dstB, in0=srcB, scalar1=1.0, scalar2=0.0,
                        op0=ALU.mult, op1=ALU.add,
                        accum_out=cb[ho:ho + 64, :])
                nc.gpsimd.dma_start(R[h:h + 64, c0:c0 + W],
                                    cmb[ho:ho + 64, 0:W],
                                    accum_op=ALU.add)

            def bisect_iter():
                tj = smallp.tile([128, 2], F32, tag="bj")
                nc.vector.tensor_scalar(out=tj[:], in0=lh[:], scalar1=0.5,
                                        scalar2=None, op0=ALU.mult,
                                        op1=ALU.add, accum_out=Tt[:])
                cnt = smallp.tile([128, 1], F32, tag="bcnt")
                nc.vector.scalar_tensor_tensor(
                    out=cjunk[:], in0=gata_s[:].rearrange("p n b -> p (n b)"),
                    scalar=Tt[:], in1=onesv[:], op0=ALU.is_lt, op1=ALU.mult,
                    accum_out=cnt[:])
                cntb = smallp.tile([128, 1], BF16, tag="bcntb")
                nc.vector.tensor_copy(cntb[:], cnt[:])
                psum_c = psf.tile([128, 512], F32, tag="bps")
                nc.tensor.matmul(psum_c[:, 0:1], onesKM[:], cntb[:],
                                 start=True, stop=True)
                m_le = smallp.tile([128, 1], I32, tag="bmle")
                m_gt = smallp.tile([128, 1], I32, tag="bmgt")
                nc.vector.tensor_scalar(out=m_le[:], in0=psum_c[:, 0:1],
                                        scalar1=float(k_sub), scalar2=None,
                                        op0=ALU.is_le)
                nc.vector.tensor_scalar(out=m_gt[:], in0=psum_c[:, 0:1],
                                        scalar1=float(k_sub), scalar2=None,
                                        op0=ALU.is_gt)
                nc.vector.copy_predicated(out=lh[:, 0:1], mask=m_le[:],
                                          data=Tt[:])
                nc.vector.copy_predicated(out=lh[:, 1:2], mask=m_gt[:],
                                          data=Tt[:])

            def p3a_pair(j, s0, ns):
                c0 = s0 * HW
                W = ns * HW
                rv = R[:, c0:c0 + W].rearrange("p (b q) -> p b q", b=ns)
                sb = sep[:, s0:s0 + ns].unsqueeze(2).broadcast_to(
                    (128, ns, HW))
                nc.vector.scalar_tensor_tensor(
                    out=rv, in0=rv, scalar=1.0, in1=sb,
                    op0=ALU.mult, op1=ALU.mult,
                    accum_out=stats[:, j:j + 1])
                sqj = rp.tile([128, GB * HW], BF16, tag="sqj")
                nc.scalar.activation(
                    sqj[:, 0:W], R[:, c0:c0 + W], AF.Square,
                    accum_out=stats[:, 32 + j:33 + j])

            def pool_pair(s0, ns):
                nc.vector.tensor_reduce(
                    out=pooled[:, s0:s0 + ns],
                    in_=resid[:, s0 * HW:(s0 + ns) * HW].rearrange(
                        "p (b q) -> p b q", b=ns),
                    axis=AX.X, op=ALU.add)

            def fc_chain():
                zp = psf.tile([128, 512], F32, tag="zfc")
                nc.tensor.matmul(zp[0:32, 0:HALF], fc1T[:],
                                 pooled[:, 0:HALF], start=True, stop=True)
                z1 = smallp.tile([32, 512], F32, tag="z1")
                nc.scalar.activation(z1[:, 0:HALF], zp[0:32, 0:HALF],
                                     AF.Relu, scale=1.0 / HW, bias=fc1b[:])
                zp2 = psf.tile([128, 512], F32, tag="zfc")
                nc.tensor.matmul(zp2[:, 0:HALF], fc2T[:],
                                 z1[:, 0:HALF], start=True, stop=True)
                nc.scalar.activation(gates[:, 0:HALF], zp2[:, 0:HALF],
                                     AF.Sigmoid, bias=fc2b[:])
                nc.sync.dma_start(ag_in[:], gates[:])
                nc.gpsimd.collective_compute(
                    "AllGather", ALU.bypass, replica_groups=rg,
                    ins=[ag_in.opt()], outs=[ag_out.opt()])
                nc.sync.dma_start(
                    gata_s[:],
                    ag_out[:, :, 0:SUB].rearrange("n p b -> p n b"))

            def make_sep():
                tj = smallp.tile([128, 2], F32, tag="bj")
                nc.vector.tensor_scalar(out=tj[:], in0=lh[:], scalar1=0.5,
                                        scalar2=None, op0=ALU.mult,
                                        op1=ALU.add, accum_out=Tt[:])
                nc.vector.tensor_scalar(out=negT[:], in0=Tt[:], scalar1=-1.0,
                                        scalar2=None, op0=ALU.mult)
                nc.scalar.activation(sep[:], gates[:], AF.Relu,
                                     bias=negT[:])

            def deferred(gi):
                for j in pool_at.get(gi, []):
                    pool_pair(pairs[j][1], pairs[j][2])
                if gi == FC_AT:
                    fc_chain()
                for _ in range(bis_at.get(gi, 0)):
                    bisect_iter()
                if gi == SEP_AT:
                    make_sep()
                for j in p3a_at.get(gi, []):
                    p3a_pair(j, pairs[j][1], pairs[j][2])

            # ---------------- conv1 loop ----------------
            NLD = (NG + 3) // 4
            for (gi, (j, par, s0, ns)) in enumerate(groups):
                slot = gi % 3
                h = 64 * par
                c0 = s0 * HW
                if gi < NLD:
                    for q in range(4 * gi, min(4 * gi + 4, NG)):
                        resid_load(q)
                # flat padded load straight from HBM
                nc.sync.dma_start(
                    xpad_f[0:64, slot, 0:ns * PADSZ],
                    x_in[:, HALF * par + s0:HALF * par + s0 + ns]
                    .rearrange("p b r w -> p (b r w)"))
                # flat-shift duplicate
                nc.sync.dma_start(
                    xpad_f[64:128, slot, 0:ns * PADSZ - 1],
                    xpad_f[0:64, slot, 1:ns * PADSZ])
                ps, tl = conv_mms(0, par, slot, ns, xpad)
                evac(0, gi, par, s0, ns, ps, tl)
                deferred(gi)

            # leftover deferred work
            for j in range(NP):
                if j not in p3a_done:
                    p3a_pair(j, pairs[j][1], pairs[j][2])

            # ---------------- BN1 allreduce ----------------
            def stats_ar(scol, qcol, slen, qt, arin, arout, cf, gcol, bcol):
                nc.vector.tensor_reduce(
                    out=sqt[:, 0:1], in_=stats[:, scol:scol + slen],
                    axis=AX.X, op=ALU.add)
                if qt is None:
                    nc.vector.tensor_reduce(
                        out=sqt[:, 1:2], in_=stats[:, qcol:qcol + slen],
                        axis=AX.X, op=ALU.add)
                else:
                    nc.vector.tensor_reduce(
                        out=sqt[:, 1:2], in_=qt[:], axis=AX.X, op=ALU.add)
                nc.gpsimd.dma_start(sqt[0:64, 0:2], sqt[64:128, 0:2],
                                    accum_op=ALU.add)
                nc.sync.dma_start(arin[:], sqt[0:64, 0:2])
                nc.gpsimd.collective_compute(
                    "AllReduce", ALU.add, replica_groups=rg,
                    ins=[arin.opt()], outs=[arout.opt()])
                sq_g = smallp.tile([C, 2], F32, tag="sqg")
                nc.sync.dma_start(sq_g[:], arout[:])
                # scratch cols: 0=mean 1=E[x^2] 2=-var 3=sd 4=isd
                nc.vector.tensor_scalar(out=scratch[:, 0:2], in0=sq_g[:],
                                        scalar1=1.0 / N1, scalar2=None,
                                        op0=ALU.mult)
                nc.vector.scalar_tensor_tensor(
                    out=scratch[:, 2:3], in0=scratch[:, 0:1],
                    scalar=scratch[:, 0:1], in1=scratch[:, 1:2],
                    op0=ALU.mult, op1=ALU.subtract)
                nc.scalar.activation(scratch[:, 3:4], scratch[:, 2:3],
                                     AF.Sqrt, scale=-1.0, bias=eps_t[:])
                nc.vector.reciprocal(scratch[:, 4:5], scratch[:, 3:4])
                nc.vector.tensor_tensor(out=cf[0:64, 0:1],
                                        in0=vecs[0:64, gcol:gcol + 1],
                                        in1=scratch[:, 4:5], op=ALU.mult)
                nc.vector.scalar_tensor_tensor(
                    out=cf[0:64, 1:2], in0=scratch[:, 0:1],
                    scalar=cf[0:64, 0:1], in1=vecs[0:64, bcol:bcol + 1],
                    op0=ALU.mult, op1=ALU.subtract)
                nc.vector.tensor_scalar(out=cf[0:64, 1:2], in0=cf[0:64, 1:2],
                                        scalar1=-1.0, scalar2=None,
                                        op0=ALU.mult)
                nc.sync.dma_start(cf[64:128, :], cf[0:64, :])

            stats_ar(0, 32, 32, None, ar1_in, ar1_out, cf1, 0, 1)

            # ---------------- conv2 loop ----------------
            for (gi, (j, par, s0, ns)) in enumerate(groups):
                slot = gi % 3
                h = 64 * par
                c0 = s0 * HW
                rv = R[h:h + 64, c0:c0 + ns * HW].rearrange(
                    "p (b r w) -> p b r w", b=ns, r=8, w=8)
                if par == 0:
                    tb = rp.tile([64, GB * HW], BF16, tag="ytmp")
                    nc.vector.tensor_scalar(
                        out=tb[:, 0:ns * HW], in0=R[0:64, c0:c0 + ns * HW],
                        scalar1=cf1[0:64, 0:1], scalar2=cf1[0:64, 1:2],
                        op0=ALU.mult, op1=ALU.add)
                    nc.vector.tensor_scalar(
                        out=ypad[0:64, slot, 0:ns, 1:9, 2:10],
                        in0=tb[:, 0:ns * HW].rearrange(
                            "p (b r w) -> p b r w", b=ns, r=8, w=8),
                        scalar1=0.0, scalar2=None, op0=ALU.max)
                else:
                    ys = (gi // 2) % 2
                    nc.scalar.activation(
                        yst[64:128, ys, 0:ns, 1:9, 2:10], rv,
                        AF.Relu, scale=cf1[64:128, 0:1],
                        bias=cf1[64:128, 1:2])
                    nc.sync.dma_start(
                        ypad_f[0:64, slot, 0:ns * PADSZ],
                        yst[:].rearrange("p s b r w -> p s (b r w)")
                        [64:128, ys, 0:ns * PADSZ])
                nc.sync.dma_start(
                    ypad_f[64:128, slot, 0:ns * PADSZ - 1],
                    ypad_f[0:64, slot, 1:ns * PADSZ])
                ps, tl = conv_mms(1, par, slot, ns, ypad)
                evac(1, gi, par, s0, ns, ps, tl)
                if par == 1:
                    # Q2 over the completed pair
                    c0p = pairs[j][1] * HW
                    Wp = pairs[j][2] * HW
                    sqj = rp.tile([128, GB * HW], BF16, tag="sqj")
                    if j % 2 == 0:
                        nc.vector.scalar_tensor_tensor(
                            out=sqj[:, 0:Wp], in0=R[:, c0p:c0p + Wp],
                            scalar=1.0, in1=R[:, c0p:c0p + Wp],
                            op0=ALU.mult, op1=ALU.mult,
                            accum_out=q2s[:, j:j + 1])
                    else:
                        nc.scalar.activation(
                            sqj[:, 0:Wp], R[:, c0p:c0p + Wp], AF.Square,
                            accum_out=q2s[:, j:j + 1])

            stats_ar(64, 0, 124, q2s, ar2_in, ar2_out, cf2, 2, 3)

            # ---------------- P5 ----------------
            p5p_cm = tc.tile_pool(name="p5p", bufs=3)
            p5p = p5p_cm.__enter__()
            p5o_cm = tc.tile_pool(name="p5o", bufs=3)
            p5o = p5o_cm.__enter__()
            for (j, s0, ns) in pairs:
                c0 = s0 * HW
                W = ns * HW
                tmp = p5p.tile([128, GB * HW], BF16, tag="p5t")
                obuf = p5o.tile([128, GB * HW], BF16, tag="obuf")
                nc.vector.scalar_tensor_tensor(
                    out=tmp[:, 0:W], in0=R[:, c0:c0 + W],
                    scalar=cf2[:, 0:1], in1=resid[:, c0:c0 + W],
                    op0=ALU.mult, op1=ALU.add)
                nc.scalar.activation(
                    obuf[:, 0:W], tmp[:, 0:W], AF.Relu, bias=cf2[:, 1:2])
                nc.sync.dma_start(
                    out_d[:, s0:s0 + ns],
                    obuf[0:64, 0:W].rearrange("p (b h w) -> p b h w",
                                              b=ns, h=8, w=8))
                nc.sync.dma_start(
                    out_d[:, HALF + s0:HALF + s0 + ns],
                    obuf[64:128, 0:W].rearrange("p (b h w) -> p b h w",
                                                b=ns, h=8, w=8))
            p5o_cm.__exit__(None, None, None)
            p5p_cm.__exit__(None, None, None)

    nc.compile()
    return nc


_NC_CACHE = {}


def _get_nc(n_cores, b_loc):
    key = (n_cores, b_loc)
    if key not in _NC_CACHE:
        _NC_CACHE[key] = build_nc(n_cores, b_loc)
    return _NC_CACHE[key]


def make_in_maps(inputs, n_cores=8):
    import ml_dtypes

    x = np.asarray(inputs["x"], dtype=np.float32)
    b_loc = x.shape[0] // n_cores

    # block-packed conv lhs: [ci, par, dy, 128, 128] -> [128, ci, par, dy, 128]
    lhsw = np.zeros((2, 2, 3, 128, 128), dtype=np.float32)
    for ci, w in ((0, inputs["conv1_w"]), (1, inputs["conv2_w"])):
        w = np.asarray(w, dtype=np.float32)
        for par in range(2):
            ma, mb = (0, 64) if par == 0 else (64, 0)
            for dy in range(3):
                lhsw[ci, par, dy, 0:64, ma:ma + 64] = w[:, :, dy, 0].T
                lhsw[ci, par, dy, 64:128, ma:ma + 64] = w[:, :, dy, 1].T
                lhsw[ci, par, dy, 64:128, mb:mb + 64] = w[:, :, dy, 2].T
    lhsw = np.ascontiguousarray(lhsw.transpose(3, 0, 1, 2, 4)).astype(
        ml_dtypes.bfloat16)

    f1 = np.asarray(inputs["fc1_w"], dtype=np.float32)    # [16, 64]
    f2 = np.asarray(inputs["fc2_w"], dtype=np.float32)    # [64, 16]
    fc1t = np.zeros((128, 32), dtype=np.float32)
    fc1t[0:64, 0:16] = f1.T
    fc1t[64:128, 16:32] = f1.T
    fc2t = np.zeros((32, 128), dtype=np.float32)
    fc2t[0:16, 0:64] = f2.T
    fc2t[16:32, 64:128] = f2.T
    fc1bp = np.tile(np.asarray(inputs["fc1_b"], np.float32), 2)[:, None]
    fc2bp = np.tile(np.asarray(inputs["fc2_b"], np.float32), 2)[:, None]
    vecsp = np.stack([np.tile(np.asarray(inputs[k], np.float32), 2)
                      for k in ("bn1_g", "bn1_b", "bn2_g", "bn2_b")], axis=1)
    wm = {"lhsw": lhsw, "fc1t": np.ascontiguousarray(fc1t),
          "fc2t": np.ascontiguousarray(fc2t), "fc1bp": fc1bp,
          "fc2bp": fc2bp, "vecsp": np.ascontiguousarray(vecsp)}

    in_maps = []
    for c in range(n_cores):
        xc = x[c * b_loc:(c + 1) * b_loc].transpose(1, 0, 2, 3)
        xr = np.ascontiguousarray(xc).astype(ml_dtypes.bfloat16)
        xp = np.zeros((64, b_loc, 10, 10), dtype=ml_dtypes.bfloat16)
        xp[:, :, 1:9, 2:10] = xr
        m = {"x": xp, "xr": xr}
        m.update(wm)
        in_maps.append(m)
    return in_maps


def kernel(**inputs):
    from concourse.bass_utils import run_bass_kernel_spmd

    x = np.asarray(inputs["x"], dtype=np.float32)
    B = x.shape[0]
    n_cores = 8
    b_loc = B // n_cores
    nc = _get_nc(n_cores, b_loc)
    in_maps = make_in_maps(inputs, n_cores)
    res = run_bass_kernel_spmd(nc, in_maps, core_ids=list(range(n_cores)))
    outs = []
    for c in range(n_cores):
        oc = np.asarray(res.results[c]["out"]).astype(np.float32)
        outs.append(oc.transpose(1, 0, 2, 3))
    return np.concatenate(outs, axis=0)


# revision 22
# speedup vs baseline: 1.0508x; 1.0121x over previous
"""Trainium2 Bass kernel for nn_BasicBlock (conv-SE-prune-BN residual block).

Data-parallel over batch across 8 NeuronCores; b_loc = 1024 per core.

v3 design (vs baseline): single x load, everything SBUF-resident.
 - Host pre-transposes x to [C, b_loc, 8, 8] and casts to bf16; output is
   returned bf16 [C, b_loc, 8, 8] and cast back on host.
 - Samples are split into two partition halves: batch 0:512 lives on
   partitions 0:64 ("L"), batch 512:1024 on 64:128 ("H"), giving 128-wide
   elementwise ops. Conv groups alternate L/H; the conv lhs has an L and
   an H variant (output accumulator halves swapped) so conv outputs land
   on their home partitions.
 - Conv: 3 matmuls per 6-sample tile, K=128 = channels + flat-shifted
   duplicate, M=128 = two accumulators (A, B). A evacuated by ACT/DVE
   copy psum->SBUF bf16; B evacuated to a bounce buffer and merged into
   R by a gpsimd DMA with accum_op=add (CCE inline add, crosses
   partitions for free).
 - Padded layout per sample is 10 rows x 10 cols, image at rows 1:9,
   cols 2:10 (so interior rows are 4-byte aligned for DVE 2x/4x modes).
   rhs views take cols 1:10; A-half out = view cols [0:8], B = [1:9].
 - Pooling from the resident bf16 copy (pair-packed, 128 partitions),
   fc gates computed pair-packed via block-packed fc weights, AllGather,
   then threshold bisection (14 iters) on a 1/8 subsample, all
   interleaved with conv1 groups. P3a (gate apply + BN1 stats) also
   interleaves with conv1's back half.
 - BN stats: per-group accum_out columns, reduced + partition-folded
   (gpsimd DMA add) + AllReduduced; coefs duplicated to both halves.
 - P5 (bn2 affine + residual + relu) runs pair-packed from SBUF.
"""
import numpy as np

import concourse.bacc as bacc
import concourse.bass as bass
import concourse.mybir as mybir
import concourse.tile as tile

F32 = mybir.dt.float32
BF16 = mybir.dt.bfloat16
I32 = mybir.dt.int32
AF = mybir.ActivationFunctionType
ALU = mybir.AluOpType
AX = mybir.AxisListType

C = 64
HW = 64
TILE_B = 6
GB = 18          # samples per conv group
PRUNE_RATE = 0.2
EPS = 1e-5
BISECT_ITERS = 10
SUB = 64         # bisect subsample columns per (core, partition)
PR, PW = 10, 10  # padded rows / cols per sample
PADSZ = PR * PW


def _pairs(half):
    """[(j, s0, ns)] covering one 512-sample half by 18-sample groups."""
    out = []
    s0 = 0
    j = 0
    while s0 < half:
        ns = min(GB, half - s0)
        out.append((j, s0, ns))
        s0 += ns
        j += 1
    return out


def _tiles(ns):
    t, b0 = [], 0
    while b0 < ns:
        nb = min(TILE_B, ns - b0)
        t.append((b0, nb))
        b0 += nb
    return t


def _transpose64(nc, dst_ap, src_ap):
    for i in (0, 32):
        for j in (0, 32):
            nc.vector.transpose(out=dst_ap[j:j + 32, i:i + 32],
                                in_=src_ap[i:i + 32, j:j + 32])


def build_nc(n_cores, b_loc):
    B_glob = n_cores * b_loc
    HALF = b_loc // 2
    N1 = float(B_glob * HW)
    n_sub = n_cores * 128 * SUB
    k_sub = PRUNE_RATE * n_sub
    D0s = float(2.0 * k_sub - n_sub)
    rg = [list(range(n_cores))]

    pairs = _pairs(HALF)          # 29 pairs
    NP = len(pairs)
    NG = 2 * NP                   # 58 groups, order L0 H0 L1 H1 ...

    nc = bacc.Bacc("TRN2", target_bir_lowering=False, debug=False,
                   enable_asserts=True, num_devices=n_cores)

    x_in = nc.dram_tensor("x", [C, b_loc, PR, PW], BF16,
                          kind="ExternalInput")
    xr_in = nc.dram_tensor("xr", [128, b_loc // 2, 8, 8], BF16,
                           kind="ExternalInput")
    lhsw_in = nc.dram_tensor("lhsw", [128, 2, 2, 3, 128], BF16,
                             kind="ExternalInput")
    fc1t_in = nc.dram_tensor("fc1t", [128, 32], F32, kind="ExternalInput")
    fc2t_in = nc.dram_tensor("fc2t", [32, 128], F32, kind="ExternalInput")
    fc1bp_in = nc.dram_tensor("fc1bp", [32, 1], F32, kind="ExternalInput")
    fc2bp_in = nc.dram_tensor("fc2bp", [128, 1], F32, kind="ExternalInput")
    vecs_in = nc.dram_tensor("vecsp", [128, 4], F32, kind="ExternalInput")
    out_d = nc.dram_tensor("out", [128, b_loc // 2, 8, 8], BF16,
                           kind="ExternalOutput")

    with tile.TileContext(nc) as tc:
        with (
            tc.tile_pool(name="persist", bufs=1) as pp,
            tc.tile_pool(name="rings", bufs=2) as rp,
            tc.tile_pool(name="small", bufs=2) as smallp,
            tc.tile_pool(name="psc", bufs=2, space="PSUM") as psc,
            tc.tile_pool(name="psf", bufs=1, space="PSUM") as psf,
            tc.tile_pool(name="dram", bufs=1, space="DRAM") as dramp,
        ):
            # ---------------- persistent SBUF ----------------
            resid = pp.tile([128, HALF * HW], BF16, tag="resid")
            R = pp.tile([128, HALF * HW], BF16, tag="R")
            xpad = pp.tile([128, 3, GB, PR, PW], BF16, tag="xpad")
            ypad = pp.tile([128, 3, GB, PR, PW], BF16, tag="ypad")
            pooled = pp.tile([128, HALF], F32, tag="pooled")
            gates = pp.tile([128, HALF], F32, tag="gates")
            sep = pp.tile([128, HALF], BF16, tag="sep")
            gata_s = pp.tile([128, n_cores, SUB], F32, tag="gata_s")
            onesKM = pp.tile([128, 128], BF16, tag="onesKM")
            stats = pp.tile([128, 192], F32, tag="stats")
            q2s = pp.tile([128, 32], F32, tag="q2s")
            vecs = pp.tile([128, 8], F32, tag="vecs")
            # vecs cols: 0=bn1_g 1=bn1_b 2=bn2_g 3=bn2_b
            fc1b = pp.tile([32, 1], F32, tag="fc1b")
            fc1T = pp.tile([128, 32], F32, tag="fc1T")
            fc2T = pp.tile([32, 128], F32, tag="fc2T")
            cf1 = pp.tile([128, 2], F32, tag="cf1")
            cf2 = pp.tile([128, 2], F32, tag="cf2")
            eps_t = pp.tile([C, 1], F32, tag="eps")
            lh = pp.tile([128, 2], F32, tag="lh")
            Tt = pp.tile([128, 1], F32, tag="Tt")
            negT = pp.tile([128, 1], F32, tag="negT")
            cjunk = pp.tile([128, n_cores * SUB], F32, tag="cjunk")
            onesv = pp.tile([128, n_cores * SUB], F32, tag="onesv")
            yst = pp.tile([128, 2, GB, PR, PW], BF16, tag="yst")
            scratch = pp.tile([C, 8], F32, tag="scratch")
            sqt = pp.tile([128, 4], F32, tag="sqt")

            xpad_f = xpad[:].rearrange("p s b r w -> p s (b r w)")
            ypad_f = ypad[:].rearrange("p s b r w -> p s (b r w)")

            # dram bounce buffers
            bar_sb = pp.tile([1, 1], F32, tag="bar_sb")
            bar_in = dramp.tile([1, 1], F32, tag="bar_in")
            bar_out = dramp.tile([1, 1], F32, tag="bar_out")
            ag_in = dramp.tile([128, SUB], F32, tag="ag_in")
            ag_out = dramp.tile([n_cores, 128, SUB], F32, tag="ag_out")
            ar1_in = dramp.tile([C, 2], F32, tag="ar1_in")
            ar1_out = dramp.tile([C, 2], F32, tag="ar1_out")
            ar2_in = dramp.tile([C, 2], F32, tag="ar2_in")
            ar2_out = dramp.tile([C, 2], F32, tag="ar2_out")

            # xpad memset first so group-0 fill can start ASAP
            nc.gpsimd.memset(xpad[:], 0)
            # early dummy collective absorbs cross-core start skew
            nc.vector.memset(bar_sb[:], 0)
            nc.sync.dma_start(bar_in[:], bar_sb[:])
            nc.gpsimd.collective_compute(
                "AllReduce", ALU.add, replica_groups=rg,
                ins=[bar_in.opt()], outs=[bar_out.opt()])

            # ---------------- constants / weights prep ----------------
            nc.gpsimd.memset(ypad[:], 0)
            nc.gpsimd.memset(yst[:], 0)
            nc.vector.memset(stats[:], 0)
            nc.vector.memset(q2s[:], 0)
            nc.vector.memset(onesKM[:], 1.0)
            nc.vector.memset(onesv[:], 1.0)
            nc.vector.memset(eps_t[:], EPS)
            nc.vector.memset(lh[:, 0:1], 0.0)
            nc.vector.memset(lh[:, 1:2], 1.0)

            lhsw = pp.tile([128, 2, 2, 3, 128], BF16, tag="lhsw")
            nc.scalar.dma_start(lhsw[:], lhsw_in[:])
            lhs = [[[lhsw[:, ci, par, dy, :] for dy in range(3)]
                    for par in range(2)] for ci in range(2)]

            nc.scalar.dma_start(fc1T[:], fc1t_in[:])
            nc.scalar.dma_start(fc2T[:], fc2t_in[:])
            nc.scalar.dma_start(fc1b[:], fc1bp_in[:])
            fc2b = pp.tile([128, 1], F32, tag="fc2b")
            nc.scalar.dma_start(fc2b[:], fc2bp_in[:])
            nc.scalar.dma_start(vecs[:, 0:4], vecs_in[:])

            # ---------------- group table ----------------
            # group gi = 2j + par ; par 0 = L (parts 0:64), 1 = H (64:128)
            groups = []
            for (j, s0, ns) in pairs:
                for par in range(2):
                    groups.append((j, par, s0, ns))

            def resid_load(j):
                (_, s0, ns) = pairs[j]
                nc.scalar.dma_start(
                    resid[:, s0 * HW:(s0 + ns) * HW],
                    xr_in[:, s0:s0 + ns].rearrange("p b h w -> p (b h w)"))

            # ---------------- deferred-work schedule ----------------
            pool_at = {}      # group -> list of pair js
            for j in range(NP):
                pool_at.setdefault(3 + j // 2, []).append(j)
            FC_AT = 3 + (NP - 1) // 2 + 1          # 18
            bis_at = {}
            g = FC_AT + 2
            for _ in range(BISECT_ITERS):
                bis_at[g] = 1
                g += 1
            SEP_AT = g             # threshold ready; make sep
            p3a_at = {}
            slot_used = {}
            for j in range(NP):
                gg = max(SEP_AT + 1 + j, 2 * j + 2)
                while gg < NG and slot_used.get(gg, 0) >= 2:
                    gg += 1
                # retry earlier second-slot if primary walk overflows
                if gg >= NG:
                    for g2 in range(max(2 * j + 2, SEP_AT + 1), NG):
                        if slot_used.get(g2, 0) < 2:
                            gg = g2
                            break
                if gg < NG:
                    p3a_at.setdefault(gg, []).append(j)
                    slot_used[gg] = slot_used.get(gg, 0) + 1
            p3a_done = {j for v in p3a_at.values() for j in v}

            # ---------------- helper ops ----------------
            def conv_mms(ci, par, slot, ns, pad):
                ps = psc.tile([128, 3, 512], F32, tag="cps")
                tl = _tiles(ns)
                for dy in range(3):
                    for (t, (tb0, nb)) in enumerate(tl):
                        nc.tensor.matmul(
                            ps[:, t, 0:nb * 72].rearrange(
                                "p (b r w) -> p b r w", b=nb, r=8, w=9),
                            lhs[ci][par][dy],
                            pad[:, slot, tb0:tb0 + nb, dy:dy + 8, 1:10],
                            start=(dy == 0), stop=(dy == 2))
                return ps, tl

            def evac(ci, gi, par, s0, ns, ps, tl):
                """A->R, B->cmb, gpsimd dma-add cmb into R."""
                h = 64 * par
                ho = 64 - h
                c0 = s0 * HW
                W = ns * HW
                cmb = rp.tile([128, GB * HW], BF16, tag="cmb")
                a_on_act = (gi % 2 == 0)
                full = (ns == GB)
                if full:
                    srcA = ps[h:h + 64, :, 0:432].rearrange(
                        "p t (b r w) -> p t b r w", b=6, r=8, w=9)[
                        :, :, :, :, 0:8]
                    srcB = ps[ho:ho + 64, :, 0:432].rearrange(
                        "p t (b r w) -> p t b r w", b=6, r=8, w=9)[
                        :, :, :, :, 1:9]
                    dstA = R[h:h + 64, c0:c0 + W].rearrange(
                        "p (t b r w) -> p t b r w", t=3, b=6, r=8, w=8)
                    dstB = cmb[ho:ho + 64, 0:W].rearrange(
                        "p (t b r w) -> p t b r w", t=3, b=6, r=8, w=8)
                else:
                    # partial group: per-tile evacs
                    for (t, (tb0, nb)) in enumerate(tl):
                        sA = ps[h:h + 64, t, 0:nb * 72].rearrange(
                            "p (b r w) -> p b r w", b=nb, r=8, w=9)[
                            :, :, :, 0:8]
                        sB = ps[ho:ho + 64, t, 0:nb * 72].rearrange(
                            "p (b r w) -> p b r w", b=nb, r=8, w=9)[
                            :, :, :, 1:9]
                        dA = R[h:h + 64, c0 + tb0 * HW:
                               c0 + (tb0 + nb) * HW].rearrange(
                            "p (b r w) -> p b r w", b=nb, r=8, w=8)
                        dB = cmb[ho:ho + 64, tb0 * HW:
                                 (tb0 + nb) * HW].rearrange(
                            "p (b r w) -> p b r w", b=nb, r=8, w=8)
                        if ci == 0:
                            nc.scalar.activation(dA, sA, AF.Copy)
                            nc.vector.tensor_copy(dB, sB)
                        else:
                            cia = 122 + (gi - 56) if t == 1 else 64 + gi
                            cib = 186 + (gi - 56) if t == 1 else 128 + gi
                            nc.scalar.activation(
                                dA, sA, AF.Copy,
                                accum_out=stats[h:h + 64, cia:cia + 1])
                            nc.vector.tensor_scalar(
                                out=dB, in0=sB, scalar1=1.0, scalar2=0.0,
                                op0=ALU.mult, op1=ALU.add,
                                accum_out=stats[ho:ho + 64, cib:cib + 1])
                    nc.gpsimd.dma_start(R[h:h + 64, c0:c0 + W],
                                        cmb[ho:ho + 64, 0:W],
                                        accum_op=ALU.add)
                    return
                if ci == 0:
                    nc.scalar.activation(dstA, srcA, AF.Copy)
                    nc.vector.tensor_copy(dstB, srcB)
                else:
                    ca = stats[:, 64 + gi:65 + gi]
                    cb = stats[:, 128 + gi:129 + gi]
                    nc.scalar.activation(dstA, srcA, AF.Copy,
                                         accum_out=ca[h:h + 64, :])
                    nc.vector.tensor_scalar(
                        out=dstB, in0=srcB, scalar1=1.0, scalar2=0.0,
                        op0=ALU.mult, op1=ALU.add,
                        accum_out=cb[ho:ho + 64, :])
                nc.gpsimd.dma_start(R[h:h + 64, c0:c0 + W],
                                    cmb[ho:ho + 64, 0:W],
                                    accum_op=ALU.add)

            def bisect_iter():
                tj = smallp.tile([128, 2], F32, tag="bj")
                nc.vector.tensor_scalar(out=tj[:], in0=lh[:], scalar1=0.5,
                                        scalar2=None, op0=ALU.mult,
                                        op1=ALU.add, accum_out=Tt[:])
                cnt = smallp.tile([128, 1], F32, tag="bcnt")
                nc.vector.scalar_tensor_tensor(
                    out=cjunk[:], in0=gata_s[:].rearrange("p n b -> p (n b)"),
                    scalar=Tt[:], in1=onesv[:], op0=ALU.is_lt, op1=ALU.mult,
                    accum_out=cnt[:])
                cntb = smallp.tile([128, 1], BF16, tag="bcntb")
                nc.vector.tensor_copy(cntb[:], cnt[:])
                psum_c = psf.tile([128, 512], F32, tag="bps")
                nc.tensor.matmul(psum_c[:, 0:1], onesKM[:], cntb[:],
                                 start=True, stop=True)
                m_le = smallp.tile([128, 1], I32, tag="bmle")
                m_gt = smallp.tile([128, 1], I32, tag="bmgt")
                nc.vector.tensor_scalar(out=m_le[:], in0=psum_c[:, 0:1],
                                        scalar1=float(k_sub), scalar2=None,
                                        op0=ALU.is_le)
                nc.vector.tensor_scalar(out=m_gt[:], in0=psum_c[:, 0:1],
                                        scalar1=float(k_sub), scalar2=None,
                                        op0=ALU.is_gt)
                nc.vector.copy_predicated(out=lh[:, 0:1], mask=m_le[:],
                                          data=Tt[:])
                nc.vector.copy_predicated(out=lh[:, 1:2], mask=m_gt[:],
                                          data=Tt[:])

            def p3a_pair(j, s0, ns):
                c0 = s0 * HW
                W = ns * HW
                rv = R[:, c0:c0 + W].rearrange("p (b q) -> p b q", b=ns)
                sb = sep[:, s0:s0 + ns].unsqueeze(2).broadcast_to(
                    (128, ns, HW))
                nc.vector.scalar_tensor_tensor(
                    out=rv, in0=rv, scalar=1.0, in1=sb,
                    op0=ALU.mult, op1=ALU.mult,
                    accum_out=stats[:, j:j + 1])
                sqj = rp.tile([128, GB * HW], BF16, tag="sqj")
                nc.scalar.activation(
                    sqj[:, 0:W], R[:, c0:c0 + W], AF.Square,
                    accum_out=stats[:, 32 + j:33 + j])

            def pool_pair(s0, ns):
                nc.vector.tensor_reduce(
                    out=pooled[:, s0:s0 + ns],
                    in_=resid[:, s0 * HW:(s0 + ns) * HW].rearrange(
                        "p (b q) -> p b q", b=ns),
                    axis=AX.X, op=ALU.add)

            def fc_chain():
                zp = psf.tile([128, 512], F32, tag="zfc")
                nc.tensor.matmul(zp[0:32, 0:HALF], fc1T[:],
                                 pooled[:, 0:HALF], start=True, stop=True)
                z1 = smallp.tile([32, 512], F32, tag="z1")
                nc.scalar.activation(z1[:, 0:HALF], zp[0:32, 0:HALF],
                                     AF.Relu, scale=1.0 / HW, bias=fc1b[:])
                zp2 = psf.tile([128, 512], F32, tag="zfc")
                nc.tensor.matmul(zp2[:, 0:HALF], fc2T[:],
                                 z1[:, 0:HALF], start=True, stop=True)
                nc.scalar.activation(gates[:, 0:HALF], zp2[:, 0:HALF],
                                     AF.Sigmoid, bias=fc2b[:])
                nc.sync.dma_start(ag_in[:], gates[:, 0:SUB])
                nc.gpsimd.collective_compute(
                    "AllGather", ALU.bypass, replica_groups=rg,
                    ins=[ag_in.opt()], outs=[ag_out.opt()])
                nc.sync.dma_start(
                    gata_s[:],
                    ag_out[:].rearrange("n p b -> p n b"))

            def make_sep():
                tj = smallp.tile([128, 2], F32, tag="bj")
                nc.vector.tensor_scalar(out=tj[:], in0=lh[:], scalar1=0.5,
                                        scalar2=None, op0=ALU.mult,
                                        op1=ALU.add, accum_out=Tt[:])
                nc.vector.tensor_scalar(out=negT[:], in0=Tt[:], scalar1=-1.0,
                                        scalar2=None, op0=ALU.mult)
                nc.scalar.activation(sep[:], gates[:], AF.Relu,
                                     bias=negT[:])

            def deferred(gi):
                for j in pool_at.get(gi, []):
                    pool_pair(pairs[j][1], pairs[j][2])
                if gi == FC_AT:
                    fc_chain()
                for _ in range(bis_at.get(gi, 0)):
                    bisect_iter()
                if gi == SEP_AT:
                    make_sep()
                for j in p3a_at.get(gi, []):
                    p3a_pair(j, pairs[j][1], pairs[j][2])

            # ---------------- conv1 loop ----------------
            NLD = (NP + 1) // 2
            for (gi, (j, par, s0, ns)) in enumerate(groups):
                slot = gi % 3
                h = 64 * par
                c0 = s0 * HW
                if gi < NLD:
                    for q in range(2 * gi, min(2 * gi + 2, NP)):
                        resid_load(q)
                # flat padded load straight from HBM
                nc.sync.dma_start(
                    xpad_f[0:64, slot, 0:ns * PADSZ],
                    x_in[:, HALF * par + s0:HALF * par + s0 + ns]
                    .rearrange("p b r w -> p (b r w)"))
                # flat-shift duplicate
                nc.sync.dma_start(
                    xpad_f[64:128, slot, 0:ns * PADSZ - 1],
                    xpad_f[0:64, slot, 1:ns * PADSZ])
                ps, tl = conv_mms(0, par, slot, ns, xpad)
                evac(0, gi, par, s0, ns, ps, tl)
                deferred(gi)

            # leftover deferred work
            for j in range(NP):
                if j not in p3a_done:
                    p3a_pair(j, pairs[j][1], pairs[j][2])

            # ---------------- BN1 allreduce ----------------
            def stats_ar(scol, qcol, slen, qt, arin, arout, cf, gcol, bcol):
                nc.vector.tensor_reduce(
                    out=sqt[:, 0:1], in_=stats[:, scol:scol + slen],
                    axis=AX.X, op=ALU.add)
                if qt is None:
                    nc.vector.tensor_reduce(
                        out=sqt[:, 1:2], in_=stats[:, qcol:qcol + slen],
                        axis=AX.X, op=ALU.add)
                else:
                    nc.vector.tensor_reduce(
                        out=sqt[:, 1:2], in_=qt[:], axis=AX.X, op=ALU.add)
                nc.gpsimd.dma_start(sqt[0:64, 0:2], sqt[64:128, 0:2],
                                    accum_op=ALU.add)
                nc.sync.dma_start(arin[:], sqt[0:64, 0:2])
                nc.gpsimd.collective_compute(
                    "AllReduce", ALU.add, replica_groups=rg,
                    ins=[arin.opt()], outs=[arout.opt()])
                sq_g = smallp.tile([C, 2], F32, tag="sqg")
                nc.sync.dma_start(sq_g[:], arout[:])
                # scratch cols: 0=mean 1=E[x^2] 2=-var 3=sd 4=isd
                nc.vector.tensor_scalar(out=scratch[:, 0:2], in0=sq_g[:],
                                        scalar1=1.0 / N1, scalar2=None,
                                        op0=ALU.mult)
                nc.vector.scalar_tensor_tensor(
                    out=scratch[:, 2:3], in0=scratch[:, 0:1],
                    scalar=scratch[:, 0:1], in1=scratch[:, 1:2],
                    op0=ALU.mult, op1=ALU.subtract)
                nc.scalar.activation(scratch[:, 3:4], scratch[:, 2:3],
                                     AF.Sqrt, scale=-1.0, bias=eps_t[:])
                nc.vector.reciprocal(scratch[:, 4:5], scratch[:, 3:4])
                nc.vector.tensor_tensor(out=cf[0:64, 0:1],
                                        in0=vecs[0:64, gcol:gcol + 1],
                                        in1=scratch[:, 4:5], op=ALU.mult)
                nc.vector.scalar_tensor_tensor(
                    out=cf[0:64, 1:2], in0=scratch[:, 0:1],
                    scalar=cf[0:64, 0:1], in1=vecs[0:64, bcol:bcol + 1],
                    op0=ALU.mult, op1=ALU.subtract)
                nc.vector.tensor_scalar(out=cf[0:64, 1:2], in0=cf[0:64, 1:2],
                                        scalar1=-1.0, scalar2=None,
                                        op0=ALU.mult)
                nc.sync.dma_start(cf[64:128, :], cf[0:64, :])

            stats_ar(0, 32, 32, None, ar1_in, ar1_out, cf1, 0, 1)

            # ---------------- conv2 loop ----------------
            for (gi, (j, par, s0, ns)) in enumerate(groups):
                slot = gi % 3
                h = 64 * par
                c0 = s0 * HW
                rv = R[h:h + 64, c0:c0 + ns * HW].rearrange(
                    "p (b r w) -> p b r w", b=ns, r=8, w=8)
                if par == 0:
                    tb = rp.tile([64, GB * HW], BF16, tag="ytmp")
                    nc.vector.tensor_scalar(
                        out=tb[:, 0:ns * HW], in0=R[0:64, c0:c0 + ns * HW],
                        scalar1=cf1[0:64, 0:1], scalar2=cf1[0:64, 1:2],
                        op0=ALU.mult, op1=ALU.add)
                    nc.vector.tensor_scalar(
                        out=ypad[0:64, slot, 0:ns, 1:9, 2:10],
                        in0=tb[:, 0:ns * HW].rearrange(
                            "p (b r w) -> p b r w", b=ns, r=8, w=8),
                        scalar1=0.0, scalar2=None, op0=ALU.max)
                else:
                    ys = (gi // 2) % 2
                    nc.scalar.activation(
                        yst[64:128, ys, 0:ns, 1:9, 2:10], rv,
                        AF.Relu, scale=cf1[64:128, 0:1],
                        bias=cf1[64:128, 1:2])
                    nc.sync.dma_start(
                        ypad_f[0:64, slot, 0:ns * PADSZ],
                        yst[:].rearrange("p s b r w -> p s (b r w)")
                        [64:128, ys, 0:ns * PADSZ])
                nc.sync.dma_start(
                    ypad_f[64:128, slot, 0:ns * PADSZ - 1],
                    ypad_f[0:64, slot, 1:ns * PADSZ])
                ps, tl = conv_mms(1, par, slot, ns, ypad)
                evac(1, gi, par, s0, ns, ps, tl)
                if par == 1:
                    # Q2 over the completed pair
                    c0p = pairs[j][1] * HW
                    Wp = pairs[j][2] * HW
                    sqj = rp.tile([128, GB * HW], BF16, tag="sqj")
                    if j % 2 == 0:
                        nc.vector.scalar_tensor_tensor(
                            out=sqj[:, 0:Wp], in0=R[:, c0p:c0p + Wp],
                            scalar=1.0, in1=R[:, c0p:c0p + Wp],
                            op0=ALU.mult, op1=ALU.mult,
                            accum_out=q2s[:, j:j + 1])
                    else:
                        nc.scalar.activation(
                            sqj[:, 0:Wp], R[:, c0p:c0p + Wp], AF.Square,
                            accum_out=q2s[:, j:j + 1])

            stats_ar(64, 0, 124, q2s, ar2_in, ar2_out, cf2, 2, 3)

            # ---------------- P5 ----------------
            p5p_cm = tc.tile_pool(name="p5p", bufs=3)
            p5p = p5p_cm.__enter__()
            p5o_cm = tc.tile_pool(name="p5o", bufs=3)
            p5o = p5o_cm.__enter__()
            for (j, s0, ns) in pairs:
                c0 = s0 * HW
                W = ns * HW
                tmp = p5p.tile([128, GB * HW], BF16, tag="p5t")
                obuf = p5o.tile([128, GB * HW], BF16, tag="obuf")
                nc.vector.scalar_tensor_tensor(
                    out=tmp[:, 0:W], in0=R[:, c0:c0 + W],
                    scalar=cf2[:, 0:1], in1=resid[:, c0:c0 + W],
                    op0=ALU.mult, op1=ALU.add)
                nc.scalar.activation(
                    obuf[:, 0:W], tmp[:, 0:W], AF.Relu, bias=cf2[:, 1:2])
                nc.sync.dma_start(
                    out_d[:, s0:s0 + ns],
                    obuf[:, 0:W].rearrange("p (b h w) -> p b h w",
                                           b=ns, h=8, w=8))
            p5o_cm.__exit__(None, None, None)
            p5p_cm.__exit__(None, None, None)

    nc.compile()
    return nc


_NC_CACHE = {}


def _get_nc(n_cores, b_loc):
    key = (n_cores, b_loc)
    if key not in _NC_CACHE:
        _NC_CACHE[key] = build_nc(n_cores, b_loc)
    return _NC_CACHE[key]


def make_in_maps(inputs, n_cores=8):
    import ml_dtypes

    x = np.asarray(inputs["x"], dtype=np.float32)
    b_loc = x.shape[0] // n_cores

    # block-packed conv lhs: [ci, par, dy, 128, 128] -> [128, ci, par, dy, 128]
    lhsw = np.zeros((2, 2, 3, 128, 128), dtype=np.float32)
    for ci, w in ((0, inputs["conv1_w"]), (1, inputs["conv2_w"])):
        w = np.asarray(w, dtype=np.float32)
        for par in range(2):
            ma, mb = (0, 64) if par == 0 else (64, 0)
            for dy in range(3):
                lhsw[ci, par, dy, 0:64, ma:ma + 64] = w[:, :, dy, 0].T
                lhsw[ci, par, dy, 64:128, ma:ma + 64] = w[:, :, dy, 1].T
                lhsw[ci, par, dy, 64:128, mb:mb + 64] = w[:, :, dy, 2].T
    lhsw = np.ascontiguousarray(lhsw.transpose(3, 0, 1, 2, 4)).astype(
        ml_dtypes.bfloat16)

    f1 = np.asarray(inputs["fc1_w"], dtype=np.float32)    # [16, 64]
    f2 = np.asarray(inputs["fc2_w"], dtype=np.float32)    # [64, 16]
    fc1t = np.zeros((128, 32), dtype=np.float32)
    fc1t[0:64, 0:16] = f1.T
    fc1t[64:128, 16:32] = f1.T
    fc2t = np.zeros((32, 128), dtype=np.float32)
    fc2t[0:16, 0:64] = f2.T
    fc2t[16:32, 64:128] = f2.T
    fc1bp = np.tile(np.asarray(inputs["fc1_b"], np.float32), 2)[:, None]
    fc2bp = np.tile(np.asarray(inputs["fc2_b"], np.float32), 2)[:, None]
    vecsp = np.stack([np.tile(np.asarray(inputs[k], np.float32), 2)
                      for k in ("bn1_g", "bn1_b", "bn2_g", "bn2_b")], axis=1)
    wm = {"lhsw": lhsw, "fc1t": np.ascontiguousarray(fc1t),
          "fc2t": np.ascontiguousarray(fc2t), "fc1bp": fc1bp,
          "fc2bp": fc2bp, "vecsp": np.ascontiguousarray(vecsp)}

    half = b_loc // 2
    in_maps = []
    for c in range(n_cores):
        xc = x[c * b_loc:(c + 1) * b_loc].transpose(1, 0, 2, 3)
        xr = np.ascontiguousarray(xc).astype(ml_dtypes.bfloat16)
        xp = np.zeros((64, b_loc, 10, 10), dtype=ml_dtypes.bfloat16)
        xp[:, :, 1:9, 2:10] = xr
        xr_pk = np.ascontiguousarray(
            np.concatenate([xr[:, :half], xr[:, half:]], axis=0))
        m = {"x": xp, "xr": xr_pk}
        m.update(wm)
        in_maps.append(m)
    return in_maps


def kernel(**inputs):
    from concourse.bass_utils import run_bass_kernel_spmd

    x = np.asarray(inputs["x"], dtype=np.float32)
    B = x.shape[0]
    n_cores = 8
    b_loc = B // n_cores
    nc = _get_nc(n_cores, b_loc)
    in_maps = make_in_maps(inputs, n_cores)
    res = run_bass_kernel_spmd(nc, in_maps, core_ids=list(range(n_cores)))
    outs = []
    for c in range(n_cores):
        oc = np.asarray(res.results[c]["out"]).astype(np.float32)
        outs.append(oc[0:64].transpose(1, 0, 2, 3))
        outs.append(oc[64:128].transpose(1, 0, 2, 3))
    return np.concatenate(outs, axis=0)
